# revision 26
# baseline (speedup 1.0000x reference)
"""CAWformer forward on 8 TRN2 NeuronCores — data parallel over batch.

Math notes (all exact algebraic rewrites of the reference):
  * irfft(xf_i * conj(xf_j)).mean(-1) == s_i * s_j / DM with s = x.sum(-1),
    so the FFT cross-correlation attention is softmax(outer(s, s)/c) @ x.
  * The 8-shift auto-attention: scores_i = <q@Wk, roll_i(x)> (+const that
    cancels in softmax); out = (sum_i p_i roll_i(x)) @ Wv.T @ Wo.T + const.
  * The depthwise smoothing conv is a (T,T) band matrix S; residual embed
    folds to inp[b].T @ (R.T @ emb_W.T) with R = I - S.
Per-core layout: rows = 2 batches x 128 channels, processed as two
128-partition chunks. Activations live rows-major (channels on partitions,
d_model on free axis); matmul operands are built feature-major via PE
transposes / algebraic tricks.
"""

import os
import numpy as np

B, T, C, DM, L, P, KS = 16, 512, 128, 512, 3, 64, 25
EPS = 1e-5
NS = DM // P           # 8 circular shifts
NC_ = 8                # cores
BPC = B // NC_         # batches per core = 2
H = 2 * DM             # FFN hidden = 1024
KD = DM // 128         # 4 k-tiles over d_model
KH = H // 128          # 8 k-tiles over hidden

F32 = None  # set lazily (mybir.dt.float32)


def _build(nc, tile, mybir, bass):
    F32 = mybir.dt.float32
    R32 = mybir.dt.float32r
    AT = mybir.ActivationFunctionType
    ALU = mybir.AluOpType
    AX = mybir.AxisListType

    def mmr(out, lhsT, rhs, start, stop):
        # float32r: single-pass PE matmul (4x the fp32 rate at N>=256),
        # fp32 memory format, ~tf32 multiply precision, fp32 accumulate.
        # Operand tiles are allocated float32r end-to-end because the BIR
        # verifier requires fp32r matmult inputs be *produced* as fp32r.
        nc.tensor.matmul(out, lhsT, rhs, start=start, stop=stop)

    # ---------------- DRAM I/O ----------------
    d = {}
    def din(name, shape, dt_=None):
        d[name] = nc.dram_tensor(name, list(shape), dt_ or F32, kind="ExternalInput")
        return d[name]

    # weight layouts are pre-shuffled on host to (128, k, n) so every DMA
    # is 128 partitions x contiguous-per-partition (full-bandwidth descriptors)
    din("xin", (128, BPC, KD, C), R32)
    din("memb", (128, KD, DM), R32)
    din("wpos", (C, DM))
    din("ident", (128, 128), R32)
    din("vw1t", (L, 128, KD, H), R32); din("vb1", (L, 128, KH)); din("vw2t", (L, 128, KH, DM), R32); din("vb2", (L, DM))
    din("aw1t", (L, 128, KD, H), R32); din("ab1", (L, 128, KH)); din("aw2t", (L, 128, KH, DM), R32); din("ab2", (L, DM))
    din("m1", (L, 128, KD, DM), R32); din("c1", (L, DM))
    din("m2", (L, 128, KD, DM), R32); din("c2", (L, DM))
    din("vsb", (L, 1)); din("asb", (L, 1))
    din("vgc", (L, C)); din("vbc", (L, C)); din("vgl", (L, DM)); din("vbl", (L, DM))
    din("agc", (L, C)); din("abc", (L, C)); din("agl", (L, DM)); din("abl", (L, DM))
    out_d = nc.dram_tensor("out", [BPC, C, DM], F32, kind="ExternalOutput")

    def bc_ap(src, parts=128):
        # broadcast a DRAM vector AP across partitions
        return bass.AP(tensor=src.tensor, offset=src.offset,
                       ap=[[0, parts]] + [list(x) for x in src.ap])

    def col_ap(src):
        # DRAM vector (n,) -> (n,1) partition-major AP
        return bass.AP(tensor=src.tensor, offset=src.offset,
                       ap=[list(src.ap[0]), [0, 1]])

    with tile.TileContext(nc) as tc:
        import contextlib
        ctx = contextlib.ExitStack()
        with ctx:
            wp = ctx.enter_context(tc.tile_pool(name="wp", bufs=1))
            ap_ = ctx.enter_context(tc.tile_pool(name="ap", bufs=1))
            bcp = ctx.enter_context(tc.tile_pool(name="bcp", bufs=9))
            sp = ctx.enter_context(tc.tile_pool(name="sp", bufs=8))
            cp = ctx.enter_context(tc.tile_pool(name="cp", bufs=1))
            pbig = ctx.enter_context(tc.tile_pool(name="pbig", bufs=3, space="PSUM"))
            ph = ctx.enter_context(tc.tile_pool(name="ph", bufs=2, space="PSUM"))
            pt = ctx.enter_context(tc.tile_pool(name="pt", bufs=2, space="PSUM"))

            # ---------------- constants ----------------
            ident = cp.tile([128, 128], R32)
            nc.sync.dma_start(out=ident, in_=d["ident"].ap())
            epsc = cp.tile([128, 1], F32)
            nc.vector.memset(epsc, EPS)
            memb_sb = cp.tile([128, KD, DM], R32)
            nc.sync.dma_start(out=memb_sb, in_=d["memb"].ap())
            wpos_sb = cp.tile([128, DM], F32)
            nc.sync.dma_start(out=wpos_sb, in_=d["wpos"].ap())
            xin_sb = cp.tile([128, BPC, KD, C], R32)
            nc.sync.dma_start(out=xin_sb, in_=d["xin"].ap())

            # ---------------- embed:  x[c] = xin[c].T @ memb + wpos ----------------
            x_t = ap_.tile([128, BPC, DM], R32, tag="x", bufs=3)
            for c in range(BPC):
                x_ps = pbig.tile([128, DM], F32, tag="big")
                for k in range(KD):
                    mmr(x_ps, xin_sb[:, c, k, :], memb_sb[:, k, :],
                        start=(k == 0), stop=(k == KD - 1))
                nc.vector.tensor_add(x_t[:, c, :], x_ps, wpos_sb)

            inv_sqc = float(1.0 / np.sqrt(DM * np.sqrt(DM)))
            phase = os.environ.get("KPHASE", "full")
            srow_of = {}

            # ---------------- layers ----------------
            for l in range(L if phase == "full" else 1):
                if phase == "emb":
                    break
                # ---- layer weight loads ----
                vw1t = wp.tile([128, KD, H], R32, tag="vw1t")
                nc.sync.dma_start(out=vw1t, in_=d["vw1t"][l])
                vb1 = sp.tile([128, KH], F32, tag="vb1")
                nc.sync.dma_start(out=vb1, in_=d["vb1"][l])
                vw2t = wp.tile([128, KH, DM], R32, tag="vw2t")
                nc.sync.dma_start(out=vw2t, in_=d["vw2t"][l])
                aw1t = wp.tile([128, KD, H], R32, tag="aw1t")
                nc.sync.dma_start(out=aw1t, in_=d["aw1t"][l])
                ab1 = sp.tile([128, KH], F32, tag="ab1")
                nc.sync.dma_start(out=ab1, in_=d["ab1"][l])
                aw2t = wp.tile([128, KH, DM], R32, tag="aw2t")
                nc.sync.dma_start(out=aw2t, in_=d["aw2t"][l])
                m1 = wp.tile([128, KD, DM], R32, tag="m1")
                nc.sync.dma_start(out=m1, in_=d["m1"][l])
                m2 = wp.tile([128, KD, DM], R32, tag="m2")
                nc.sync.dma_start(out=m2, in_=d["m2"][l])

                vgc = sp.tile([128, 1], F32, tag="vgc")
                nc.gpsimd.dma_start(out=vgc, in_=col_ap(d["vgc"][l]))
                agc = sp.tile([128, 1], F32, tag="agc")
                nc.gpsimd.dma_start(out=agc, in_=col_ap(d["agc"][l]))
                vbc = sp.tile([128, 1], F32, tag="vbc")
                nc.gpsimd.dma_start(out=vbc, in_=col_ap(d["vbc"][l]))
                abc = sp.tile([128, 1], F32, tag="abc")
                nc.gpsimd.dma_start(out=abc, in_=col_ap(d["abc"][l]))

                def bcast(name):
                    t = bcp.tile([128, DM], F32, tag="bc", name=f"{name}_bc{l}")
                    nc.gpsimd.dma_start(out=t, in_=bc_ap(d[name][l]))
                    return t
                c1b = bcast("c1"); c2b = bcast("c2")
                vb2b = bcast("vb2"); ab2b = bcast("ab2")
                vglb = bcast("vgl"); vblb = bcast("vbl")
                aglb = bcast("agl"); ablb = bcast("abl")

                # gcI = diag(gc_vc) as dense tile for the "+I" residual fold
                gcI = sp.tile([128, 128], F32, tag="gcI", bufs=2)
                nc.gpsimd.tensor_scalar_mul(gcI, ident, vgc)
                vsb = sp.tile([128, 1], F32, tag="vsb")
                nc.gpsimd.dma_start(out=vsb, in_=bc_ap(d["vsb"][l]))
                asb = sp.tile([128, 1], F32, tag="asb")
                nc.gpsimd.dma_start(out=asb, in_=bc_ap(d["asb"][l]))

                # ============ VarCor block ============
                # s = rowsum(x) * 1/sqrt(DM*sqrt(DM)) (split sqrt per side)
                cT = ap_.tile([128, BPC, 128], R32, tag="cT")
                for c in range(BPC):
                    if c in srow_of:
                        srow = srow_of[c]
                    else:
                        srow = sp.tile([128, 1], F32, tag="srow", bufs=4)
                        nc.vector.tensor_reduce(srow, x_t[:, c, :], AX.X, ALU.add)
                    s2 = sp.tile([128, 1], R32, tag="s2", bufs=4)
                    nc.scalar.mul(s2, srow, inv_sqc)
                    sT_ps = pbig.tile([1, 128], R32, tag="big", name=f"sTps{l}_{c}")
                    nc.tensor.transpose(sT_ps, s2, ident)
                    sT = sp.tile([1, 128], R32, tag="sT", bufs=4)
                    nc.scalar.activation(sT, sT_ps, AT.Identity)
                    corr_ps = pbig.tile([128, 128], F32, tag="big", name=f"corrps{l}_{c}")
                    mmr(corr_ps, sT, sT, start=True, stop=True)
                    # softmax over free axis (values are O(1): skip max-sub)
                    # + BN row-scale + +I fold
                    corrE = ap_.tile([128, 128], F32, tag="corrE", bufs=2)
                    rsum = sp.tile([128, 1], F32, tag="rsum", bufs=4)
                    nc.scalar.activation(corrE, corr_ps, AT.Exp, accum_out=rsum)
                    rinv = sp.tile([128, 1], F32, tag="rinv", bufs=4)
                    nc.vector.reciprocal(rinv, rsum)
                    corrBN = ap_.tile([128, 128], R32, tag="corrBN", bufs=2)
                    nc.vector.tensor_scalar(corrBN, corrE, rinv, vgc, ALU.mult, ALU.mult)
                    nc.vector.tensor_add(corrBN, corrBN, gcI)
                    cT_ps = pt.tile([128, 128], R32, tag="t", name=f"cTps{l}_{c}")
                    nc.tensor.transpose(cT_ps, corrBN, ident)
                    nc.vector.tensor_copy(cT[:, c, :], cT_ps)

                # r2 rows-major and feature-major via two matmul sets
                vbcf = bcp.tile([128, 128], F32, tag="bc2", bufs=4, name=f"vbcf{l}")
                nc.gpsimd.dma_start(out=vbcf, in_=bc_ap(d["vbc"][l]))
                r2r = ap_.tile([128, BPC, DM], R32, tag="r2r")
                r2T = ap_.tile([128, KD, 2 * 128], R32, tag="r2T")
                for c in range(BPC):
                    rr_ps = pbig.tile([128, DM], F32, tag="big", name=f"rrps{l}_{c}")
                    mmr(rr_ps, cT[:, c, :], x_t[:, c, :], start=True, stop=True)
                    nc.scalar.activation(r2r[:, c, :], rr_ps, AT.Identity, bias=vbc)
                    for m in range(KD):
                        rt_ps = pt.tile([128, 128], F32, tag="t", name=f"rtps{l}_{c}_{m}")
                        mmr(rt_ps, x_t[:, c, m * 128:(m + 1) * 128],
                            cT[:, c, :], start=True, stop=True)
                        # feature-major r2T: BN beta is along the free (channel)
                        # axis here, so add it via a partition-broadcast tile
                        nc.vector.tensor_add(r2T[:, m, c * 128:(c + 1) * 128],
                                             rt_ps, vbcf)

                if phase == "corr":
                    x_t = r2r
                    break
                x_t = _ffn_ln(nc, tile, mybir, bass, tc, ap_, sp, bcp, ph, pbig,
                              r2T, r2r, vw1t, vb1, vw2t, vb2b, vglb, vblb, l, "v", epsc,
                              vsb, inv_sqc, srow_of)
                if phase == "vc0":
                    break

                # ============ Auto-attention block ============
                # xT feature-major
                xT = ap_.tile([128, KD, 2 * 128], R32, tag="xT")
                for c in range(BPC):
                    for m in range(KD):
                        tp = pt.tile([128, 128], R32, tag="t", name=f"xTps{l}_{c}_{m}")
                        nc.tensor.transpose(tp, x_t[:, c, m * 128:(m + 1) * 128], ident)
                        nc.vector.tensor_copy(xT[:, m, c * 128:(c + 1) * 128], tp)

                # u = x @ M1 + c1   (rows-major out)
                u_t = ap_.tile([128, BPC, DM], F32, tag="u")
                for c in range(BPC):
                    u_ps = pbig.tile([128, DM], F32, tag="big", name=f"ups{l}_{c}")
                    for k in range(KD):
                        mmr(u_ps, xT[:, k, c * 128:(c + 1) * 128],
                            m1[:, k, :], start=(k == 0), stop=(k == KD - 1))
                    nc.vector.tensor_add(u_t[:, c, :], u_ps, c1b)

                if phase == "u":
                    x_t = u_t
                    break

                # scores S[r,i] = <u, roll_i(x)> * DM^-0.5 ; softmax over i
                scl = float(DM ** -0.5)
                Sp_t = ap_.tile([128, BPC, NS], F32, tag="Sp")
                trash = ap_.tile([128, DM], F32, tag="trash", bufs=1)
                for c in range(BPC):
                    # NOTE: tensor_tensor_reduce wedges the device on this
                    # walrus/NRT build (NRT_EXEC_UNIT_UNRECOVERABLE); use
                    # scalar_tensor_tensor's accum_out instead.
                    Sa = sp.tile([128, NS], F32, tag="Sa", bufs=2)
                    Sb = sp.tile([128, NS], F32, tag="Sb", bufs=2)
                    nc.gpsimd.memset(Sb[:, 0:1], 0.0)
                    for i in range(NS):
                        sh = P * i
                        if sh == 0:
                            nc.vector.scalar_tensor_tensor(
                                out=trash, in0=u_t[:, c, :], scalar=scl,
                                in1=x_t[:, c, :], op0=ALU.mult, op1=ALU.mult,
                                accum_out=Sa[:, i:i + 1])
                        else:
                            nc.vector.scalar_tensor_tensor(
                                out=trash[:, :DM - sh], in0=u_t[:, c, :DM - sh],
                                scalar=scl, in1=x_t[:, c, sh:],
                                op0=ALU.mult, op1=ALU.mult, accum_out=Sa[:, i:i + 1])
                            nc.vector.scalar_tensor_tensor(
                                out=trash[:, DM - sh:], in0=u_t[:, c, DM - sh:],
                                scalar=scl, in1=x_t[:, c, :sh],
                                op0=ALU.mult, op1=ALU.mult, accum_out=Sb[:, i:i + 1])
                    S = sp.tile([128, NS], F32, tag="S", bufs=2)
                    nc.gpsimd.tensor_add(S, Sa, Sb)
                    Se = sp.tile([128, NS], F32, tag="Se", bufs=2)
                    ssum = sp.tile([128, 1], F32, tag="ssum", bufs=4)
                    nc.scalar.activation(Se, S, AT.Exp, accum_out=ssum)
                    sinv = sp.tile([128, 1], F32, tag="sinv", bufs=4)
                    nc.vector.reciprocal(sinv, ssum)
                    nc.vector.tensor_scalar(Sp_t[:, c, :], Se, sinv, None, ALU.mult)

                if phase == "sc":
                    x_t = ap_.tile([128, BPC, DM], F32, tag="scdump")
                    nc.vector.memset(x_t, 0.0)
                    for c in range(BPC):
                        nc.vector.tensor_copy(x_t[:, c, 0:NS], Sp_t[:, c, :])
                    break

                # vm = sum_i p_i roll_i(x) via diag matmuls accumulating in PSUM
                vm_t = ap_.tile([128, BPC, DM], R32, tag="vm")
                for c in range(BPC):
                    vm_ps = pbig.tile([128, DM], F32, tag="big", name=f"vmps{l}_{c}")
                    for i in range(NS):
                        dg = ap_.tile([128, 128], R32, tag="dg", bufs=3)
                        nc.gpsimd.tensor_scalar_mul(dg, ident, Sp_t[:, c, i:i + 1])
                        sh = P * i
                        last = (i == NS - 1)
                        if sh == 0:
                            mmr(vm_ps, dg, x_t[:, c, :], start=True, stop=False)
                        else:
                            mmr(vm_ps[:, :DM - sh], dg, x_t[:, c, sh:],
                                start=False, stop=False)
                            mmr(vm_ps[:, DM - sh:], dg, x_t[:, c, :sh],
                                start=False, stop=last)
                    nc.scalar.activation(vm_t[:, c, :], vm_ps, AT.Identity)

                if phase == "vm":
                    x_t = vm_t
                    break

                # vmT feature-major
                vmT = ap_.tile([128, KD, 2 * 128], R32, tag="vmT")
                for c in range(BPC):
                    for m in range(KD):
                        tp2 = pt.tile([128, 128], R32, tag="t", name=f"vmTps{l}_{c}_{m}")
                        nc.tensor.transpose(tp2, vm_t[:, c, m * 128:(m + 1) * 128], ident)
                        nc.scalar.activation(vmT[:, m, c * 128:(c + 1) * 128], tp2, AT.Identity)

                # attn out rows-major: o = vm @ M2 + c2 ; r1 = BN(o + x)
                r1r = ap_.tile([128, BPC, DM], R32, tag="r1r")
                for c in range(BPC):
                    o_ps = pbig.tile([128, DM], F32, tag="big", name=f"ops{l}_{c}")
                    for k in range(KD):
                        mmr(o_ps, vmT[:, k, c * 128:(c + 1) * 128],
                            m2[:, k, :], start=(k == 0), stop=(k == KD - 1))
                    t1 = ap_.tile([128, DM], F32, tag="t1", bufs=2)
                    nc.vector.tensor_add(t1, o_ps, x_t[:, c, :])
                    nc.vector.tensor_add(t1, t1, c2b)
                    nc.scalar.activation(r1r[:, c, :], t1, AT.Identity, bias=abc, scale=agc)

                if phase == "attn":
                    x_t = r1r
                    break

                # r1T feature-major
                r1T = ap_.tile([128, KD, 2 * 128], R32, tag="r1T")
                for c in range(BPC):
                    for m in range(KD):
                        tp3 = pt.tile([128, 128], R32, tag="t", name=f"r1Tps{l}_{c}_{m}")
                        nc.tensor.transpose(tp3, r1r[:, c, m * 128:(m + 1) * 128], ident)
                        nc.scalar.activation(r1T[:, m, c * 128:(c + 1) * 128], tp3, AT.Identity)

                x_t = _ffn_ln(nc, tile, mybir, bass, tc, ap_, sp, bcp, ph, pbig,
                              r1T, r1r, aw1t, ab1, aw2t, ab2b, aglb, ablb, l, "a", epsc,
                              asb, inv_sqc, srow_of)

            # ---------------- store ----------------
            for c in range(BPC):
                nc.sync.dma_start(out=out_d.ap()[c], in_=x_t[:, c, :].bitcast(F32))


def _ffn_ln(nc, tile, mybir, bass, tc, ap_, sp, bcp, ph, pbig,
            rT, rrows, w1t, b1, w2t, b2b, glb, blb, l, pfx, epsc,
            sumb, inv_sqc, srow_of):
    """h = gelu(r @ W1.T + b1); y = h @ W2.T + b2; x = LN(y + r) * g + b.

    Also emits (for the "a" blocks feeding the next varcor) the row-sum of
    the next x via <xn, g> + sum(b) so the correlation chain never waits on
    the gamma/beta affine (which runs on GpSimd off the critical path)."""
    F32 = mybir.dt.float32
    R32 = mybir.dt.float32r
    AT = mybir.ActivationFunctionType
    ALU = mybir.AluOpType
    AX = mybir.AxisListType

    hT = ap_.tile([128, KH, 2 * 128], R32, tag="hT", bufs=1, name=f"hT{pfx}{l}")
    for mh2 in range(KH // 2):
        h_ps = ph.tile([128, 2, 128 * 2], F32, tag="h", name=f"hps{pfx}{l}_{mh2}")
        for half in range(2):
            mh = mh2 * 2 + half
            for k in range(KD):
                nc.tensor.matmul(h_ps[:, half, :], w1t[:, k, mh * 128:(mh + 1) * 128],
                                 rT[:, k, :], start=(k == 0), stop=(k == KD - 1))
            nc.scalar.activation(hT[:, mh, :], h_ps[:, half, :], AT.Gelu,
                                 bias=b1[:, mh:mh + 1])

    x_new = ap_.tile([128, BPC, DM], R32, tag="x", bufs=3, name=f"x{pfx}{l}")
    for c in range(BPC):
        y_ps = pbig.tile([128, DM], F32, tag="big", name=f"yps{pfx}{l}_{c}")
        for k in range(KH):
            nc.tensor.matmul(y_ps, hT[:, k, c * 128:(c + 1) * 128],
                             w2t[:, k, :], start=(k == 0), stop=(k == KH - 1))
        rb = ap_.tile([128, DM], F32, tag="rb", bufs=2, name=f"rb{pfx}{l}_{c}")
        nc.gpsimd.tensor_add(rb, rrows[:, c, :], b2b)
        z = ap_.tile([128, DM], F32, tag="z", bufs=2, name=f"z{pfx}{l}_{c}")
        nc.vector.tensor_add(z, y_ps, rb)
        st6 = sp.tile([128, 6], F32, tag="st6", bufs=4)
        nc.vector.bn_stats(out=st6, in_=z)
        mv = sp.tile([128, 2], F32, tag="mv", bufs=4)
        nc.vector.bn_aggr(out=mv, in_=st6)
        std = sp.tile([128, 1], F32, tag="std", bufs=4)
        nc.scalar.activation(std, mv[:, 1:2], AT.Sqrt, bias=epsc)
        rstd = sp.tile([128, 1], F32, tag="rstd", bufs=4)
        nc.vector.reciprocal(rstd, std)
        nb = sp.tile([128, 1], F32, tag="nb", bufs=4)
        nc.vector.tensor_scalar_mul(nb, mv[:, 0:1], -1.0)
        xn = ap_.tile([128, DM], F32, tag="xn", bufs=2, name=f"xn{pfx}{l}_{c}")
        nc.vector.tensor_scalar(xn, z, nb, rstd, ALU.add, ALU.mult)
        if pfx == "a" and l < L - 1:
            # next-layer corr row-sum: <xn*rstd-normalized x, g> + sum(b)
            trash2 = ap_.tile([128, DM], F32, tag="trash", bufs=1,
                              name=f"tr2{pfx}{l}_{c}")
            sraw = sp.tile([128, 1], F32, tag="sraw", bufs=4)
            nc.vector.scalar_tensor_tensor(
                out=trash2, in0=xn, scalar=1.0, in1=glb,
                op0=ALU.mult, op1=ALU.mult, accum_out=sraw)
            srow = sp.tile([128, 1], F32, tag="srow", bufs=4, name=f"srow{pfx}{l}_{c}")
            nc.scalar.activation(srow, sraw, AT.Identity, bias=sumb)
            srow_of[c] = srow
        # affine off the critical corr path (GpSimd), in place
        nc.gpsimd.tensor_mul(x_new[:, c, :], xn, glb)
        nc.gpsimd.tensor_add(x_new[:, c, :], x_new[:, c, :], blb)
    return x_new


# ======================================================================
# host side
# ======================================================================

_COMPILED = {}


def _compile():
    if "nc" in _COMPILED:
        return _COMPILED["nc"]
    import concourse.bass as bass
    import concourse.bacc as bacc
    import concourse.tile as tile
    from concourse import mybir
    nc = bacc.Bacc("TRN2", target_bir_lowering=False, debug=False, num_devices=NC_)
    _build(nc, tile, mybir, bass)
    nc.compile()
    _COMPILED["nc"] = nc
    return nc


def _host_prep(inputs):
    f = lambda k: np.asarray(inputs[k], np.float32)
    ld_w = f("ld_w").reshape(KS).astype(np.float64)
    # conv matrix with replicate padding, R = I - S
    S = np.zeros((T, T), np.float64)
    idx = np.clip(np.arange(T)[:, None] + np.arange(KS)[None, :] - KS // 2, 0, T - 1)
    for k in range(KS):
        np.add.at(S, (np.arange(T), idx[:, k]), ld_w[k])
    Rm = np.eye(T) - S
    emb_W = f("emb_W").astype(np.float64)
    memb = (Rm.T @ emb_W.T).astype(np.float32)              # (T, DM)
    wpos = (f("W_pos") + f("emb_b")[None, :]
            - float(f("ld_b")[0]) * emb_W.sum(1).astype(np.float32)[None, :])

    g = {"memb": np.ascontiguousarray(memb.reshape(KD, 128, DM).transpose(1, 0, 2)),
         "wpos": np.ascontiguousarray(wpos.astype(np.float32)),
         "ident": np.eye(128, dtype=np.float32)}

    s1 = np.float32(1.0 / np.sqrt(1.0 + EPS))
    def stack(fn):
        return np.ascontiguousarray(np.stack([fn(l) for l in range(L)]).astype(np.float32))

    def shuf(a):
        # (k*128, n) -> (128, k, n): SBUF layout with contiguous per-partition rows
        kn, n = a.shape
        return a.reshape(kn // 128, 128, n).transpose(1, 0, 2)

    g["vw1t"] = stack(lambda l: shuf(f("vc_W1")[l].T))
    g["vb1"] = stack(lambda l: f("vc_b1")[l].reshape(KH, 128).T)
    g["vw2t"] = stack(lambda l: shuf(f("vc_W2")[l].T))
    g["vb2"] = stack(lambda l: f("vc_b2")[l])
    g["aw1t"] = stack(lambda l: shuf(f("aa_W1")[l].T))
    g["ab1"] = stack(lambda l: f("aa_b1")[l].reshape(KH, 128).T)
    g["aw2t"] = stack(lambda l: shuf(f("aa_W2")[l].T))
    g["ab2"] = stack(lambda l: f("aa_b2")[l])
    g["m1"] = stack(lambda l: shuf(f("aa_Wq")[l].astype(np.float64).T @ f("aa_Wk")[l].astype(np.float64)))
    g["c1"] = stack(lambda l: f("aa_bq")[l].astype(np.float64) @ f("aa_Wk")[l].astype(np.float64))
    g["m2"] = stack(lambda l: shuf((f("aa_Wo")[l].astype(np.float64) @ f("aa_Wv")[l].astype(np.float64)).T))
    g["c2"] = stack(lambda l: f("aa_bv")[l].astype(np.float64) @ f("aa_Wo")[l].astype(np.float64).T
                    + f("aa_bo")[l].astype(np.float64))
    g["vsb"] = stack(lambda l: f("vc_ln_b")[l].sum(keepdims=True))
    g["asb"] = stack(lambda l: f("aa_ln_b")[l].sum(keepdims=True))
    g["vgc"] = stack(lambda l: f("vc_bn_g")[l] * s1)
    g["vbc"] = stack(lambda l: f("vc_bn_b")[l])
    g["vgl"] = stack(lambda l: f("vc_ln_g")[l])
    g["vbl"] = stack(lambda l: f("vc_ln_b")[l])
    g["agc"] = stack(lambda l: f("aa_bn_g")[l] * s1)
    g["abc"] = stack(lambda l: f("aa_bn_b")[l])
    g["agl"] = stack(lambda l: f("aa_ln_g")[l])
    g["abl"] = stack(lambda l: f("aa_ln_b")[l])
    return g


def kernel(**inputs):
    from concourse.bass_utils import run_bass_kernel_spmd
    nc = _compile()
    g = _host_prep(inputs)
    inp = np.asarray(inputs["inp"], np.float32)
    in_maps = []
    for core in range(NC_):
        m = dict(g)
        sl = inp[core * BPC:(core + 1) * BPC]          # (BPC, T, C)
        m["xin"] = np.ascontiguousarray(
            sl.reshape(BPC, KD, 128, C).transpose(2, 0, 1, 3))
        in_maps.append(m)
    res = run_bass_kernel_spmd(nc, in_maps, core_ids=list(range(NC_)))
    if res.exec_time_ns is not None:
        kernel.last_exec_time_ns = res.exec_time_ns
    out = np.concatenate([res.results[k]["out"] for k in range(NC_)], axis=0)
    return out


kernel.last_exec_time_ns = None


# revision 27
# speedup vs baseline: 1.1375x; 1.1375x over previous
"""CAWformer forward on 8 TRN2 NeuronCores — data parallel over batch.

Math notes (all exact algebraic rewrites of the reference):
  * irfft(xf_i * conj(xf_j)).mean(-1) == s_i * s_j / DM with s = x.sum(-1),
    so the FFT cross-correlation attention is softmax(outer(s, s)/c) @ x.
  * The 8-shift auto-attention: scores_i = <q@Wk, roll_i(x)> (+const that
    cancels in softmax); out = (sum_i p_i roll_i(x)) @ Wv.T @ Wo.T + const.
  * The depthwise smoothing conv is a (T,T) band matrix S; residual embed
    folds to inp[b].T @ (R.T @ emb_W.T) with R = I - S.
Per-core layout: rows = 2 batches x 128 channels, processed as two
128-partition chunks. Activations live rows-major (channels on partitions,
d_model on free axis); matmul operands are built feature-major via PE
transposes / algebraic tricks.
"""

import os
import numpy as np

B, T, C, DM, L, P, KS = 16, 512, 128, 512, 3, 64, 25
EPS = 1e-5
NS = DM // P           # 8 circular shifts
NC_ = 8                # cores
BPC = B // NC_         # batches per core = 2
H = 2 * DM             # FFN hidden = 1024
KD = DM // 128         # 4 k-tiles over d_model
KH = H // 128          # 8 k-tiles over hidden

F32 = None  # set lazily (mybir.dt.float32)


def _build(nc, tile, mybir, bass):
    F32 = mybir.dt.float32
    R32 = mybir.dt.float32r
    AT = mybir.ActivationFunctionType
    ALU = mybir.AluOpType
    AX = mybir.AxisListType

    def mmr(out, lhsT, rhs, start, stop):
        # float32r: single-pass PE matmul (4x the fp32 rate at N>=256),
        # fp32 memory format, ~tf32 multiply precision, fp32 accumulate.
        # Operand tiles are allocated float32r end-to-end because the BIR
        # verifier requires fp32r matmult inputs be *produced* as fp32r.
        nc.tensor.matmul(out, lhsT, rhs, start=start, stop=stop)

    # ---------------- DRAM I/O ----------------
    d = {}
    def din(name, shape, dt_=None):
        d[name] = nc.dram_tensor(name, list(shape), dt_ or F32, kind="ExternalInput")
        return d[name]

    # weight layouts are pre-shuffled on host to (128, k, n) so every DMA
    # is 128 partitions x contiguous-per-partition (full-bandwidth descriptors)
    din("xin", (128, BPC, KD, C), R32)
    din("memb", (128, KD, DM), R32)
    din("wpos", (C, DM))
    din("ident", (128, 128), R32)
    din("vw1t", (L, 128, KD, H), R32); din("vb1", (L, 128, KH)); din("vw2t", (L, 128, KH, DM), R32); din("vb2", (L, DM))
    din("aw1t", (L, 128, KD, H), R32); din("ab1", (L, 128, KH)); din("aw2t", (L, 128, KH, DM), R32); din("ab2", (L, DM))
    din("m1", (L, 128, KD, DM), R32); din("c1", (L, DM))
    din("m2", (L, 128, KD, DM), R32); din("c2", (L, DM))
    din("vsb", (L, 1)); din("asb", (L, 1))
    din("vgc", (L, C)); din("vbc", (L, C)); din("vgl", (L, DM)); din("vbl", (L, DM))
    din("agc", (L, C)); din("abc", (L, C)); din("agl", (L, DM)); din("abl", (L, DM))
    out_d = nc.dram_tensor("out", [BPC, C, DM], F32, kind="ExternalOutput")

    def bc_ap(src, parts=128):
        # broadcast a DRAM vector AP across partitions
        return bass.AP(tensor=src.tensor, offset=src.offset,
                       ap=[[0, parts]] + [list(x) for x in src.ap])

    def col_ap(src):
        # DRAM vector (n,) -> (n,1) partition-major AP
        return bass.AP(tensor=src.tensor, offset=src.offset,
                       ap=[list(src.ap[0]), [0, 1]])

    with tile.TileContext(nc) as tc:
        import contextlib
        ctx = contextlib.ExitStack()
        with ctx:
            wp = ctx.enter_context(tc.tile_pool(name="wp", bufs=1))
            ap_ = ctx.enter_context(tc.tile_pool(name="ap", bufs=1))
            bcp = ctx.enter_context(tc.tile_pool(name="bcp", bufs=9))
            sp = ctx.enter_context(tc.tile_pool(name="sp", bufs=8))
            cp = ctx.enter_context(tc.tile_pool(name="cp", bufs=1))
            pbig = ctx.enter_context(tc.tile_pool(name="pbig", bufs=3, space="PSUM"))
            ph = ctx.enter_context(tc.tile_pool(name="ph", bufs=2, space="PSUM"))
            pt = ctx.enter_context(tc.tile_pool(name="pt", bufs=2, space="PSUM"))

            # ---------------- constants ----------------
            ident = cp.tile([128, 128], R32)
            nc.sync.dma_start(out=ident, in_=d["ident"].ap())
            epsc = cp.tile([128, 1], F32)
            nc.vector.memset(epsc, EPS)
            memb_sb = cp.tile([128, KD, DM], R32)
            nc.sync.dma_start(out=memb_sb, in_=d["memb"].ap())
            wpos_sb = cp.tile([128, DM], F32)
            nc.sync.dma_start(out=wpos_sb, in_=d["wpos"].ap())
            xin_sb = cp.tile([128, BPC, KD, C], R32)
            nc.sync.dma_start(out=xin_sb, in_=d["xin"].ap())

            # ---------------- embed:  x[c] = xin[c].T @ memb + wpos ----------------
            x_t = ap_.tile([128, BPC, DM], R32, tag="x", bufs=3)
            for c in range(BPC):
                x_ps = pbig.tile([128, DM], F32, tag="big")
                for k in range(KD):
                    mmr(x_ps, xin_sb[:, c, k, :], memb_sb[:, k, :],
                        start=(k == 0), stop=(k == KD - 1))
                nc.vector.tensor_add(x_t[:, c, :], x_ps, wpos_sb)

            inv_sqc = float(1.0 / np.sqrt(DM * np.sqrt(DM)))
            phase = os.environ.get("KPHASE", "full")
            srow_of = {}

            # ---------------- layers ----------------
            for l in range(L if phase == "full" else 1):
                if phase == "emb":
                    break
                # ---- layer weight loads ----
                vw1t = wp.tile([128, KD, H], R32, tag="vw1t")
                nc.sync.dma_start(out=vw1t, in_=d["vw1t"][l])
                vb1 = sp.tile([128, KH], F32, tag="vb1")
                nc.sync.dma_start(out=vb1, in_=d["vb1"][l])
                vw2t = wp.tile([128, KH, DM], R32, tag="vw2t")
                nc.sync.dma_start(out=vw2t, in_=d["vw2t"][l])
                aw1t = wp.tile([128, KD, H], R32, tag="aw1t")
                nc.sync.dma_start(out=aw1t, in_=d["aw1t"][l])
                ab1 = sp.tile([128, KH], F32, tag="ab1")
                nc.sync.dma_start(out=ab1, in_=d["ab1"][l])
                aw2t = wp.tile([128, KH, DM], R32, tag="aw2t")
                nc.sync.dma_start(out=aw2t, in_=d["aw2t"][l])
                m1 = wp.tile([128, KD, DM], R32, tag="m1")
                nc.sync.dma_start(out=m1, in_=d["m1"][l])
                m2 = wp.tile([128, KD, DM], R32, tag="m2")
                nc.sync.dma_start(out=m2, in_=d["m2"][l])

                vgc = sp.tile([128, 1], F32, tag="vgc")
                nc.gpsimd.dma_start(out=vgc, in_=col_ap(d["vgc"][l]))
                agc = sp.tile([128, 1], F32, tag="agc")
                nc.gpsimd.dma_start(out=agc, in_=col_ap(d["agc"][l]))
                vbc = sp.tile([128, 1], F32, tag="vbc")
                nc.gpsimd.dma_start(out=vbc, in_=col_ap(d["vbc"][l]))
                abc = sp.tile([128, 1], F32, tag="abc")
                nc.gpsimd.dma_start(out=abc, in_=col_ap(d["abc"][l]))

                def bcast(name):
                    t = bcp.tile([128, DM], F32, tag="bc", name=f"{name}_bc{l}")
                    nc.gpsimd.dma_start(out=t, in_=bc_ap(d[name][l]))
                    return t
                c1b = bcast("c1"); c2b = bcast("c2")
                vb2b = bcast("vb2"); ab2b = bcast("ab2")
                vglb = bcast("vgl"); vblb = bcast("vbl")
                aglb = bcast("agl"); ablb = bcast("abl")

                # gcI = diag(gc_vc) as dense tile for the "+I" residual fold
                gcI = sp.tile([128, 128], F32, tag="gcI", bufs=2)
                nc.vector.tensor_scalar_mul(gcI, ident, vgc)
                vsb = sp.tile([128, 1], F32, tag="vsb")
                nc.gpsimd.dma_start(out=vsb, in_=bc_ap(d["vsb"][l]))
                asb = sp.tile([128, 1], F32, tag="asb")
                nc.gpsimd.dma_start(out=asb, in_=bc_ap(d["asb"][l]))

                # ============ VarCor block ============
                # s = rowsum(x) * 1/sqrt(DM*sqrt(DM)) (split sqrt per side)
                cT = ap_.tile([128, BPC, 128], R32, tag="cT")
                for c in range(BPC):
                    if c in srow_of:
                        srow = srow_of[c]
                    else:
                        srow = sp.tile([128, 1], F32, tag="srow", bufs=4)
                        nc.vector.tensor_reduce(srow, x_t[:, c, :], AX.X, ALU.add)
                    s2 = sp.tile([128, 1], R32, tag="s2", bufs=4)
                    nc.scalar.mul(s2, srow, inv_sqc)
                    sT_ps = pbig.tile([1, 128], R32, tag="big", name=f"sTps{l}_{c}")
                    nc.tensor.transpose(sT_ps, s2, ident)
                    sT = sp.tile([1, 128], R32, tag="sT", bufs=4)
                    nc.scalar.activation(sT, sT_ps, AT.Identity)
                    corr_ps = pbig.tile([128, 128], F32, tag="big", name=f"corrps{l}_{c}")
                    mmr(corr_ps, sT, sT, start=True, stop=True)
                    # softmax over free axis (values are O(1): skip max-sub)
                    # + BN row-scale + +I fold
                    corrE = ap_.tile([128, 128], F32, tag="corrE", bufs=2)
                    rsum = sp.tile([128, 1], F32, tag="rsum", bufs=4)
                    nc.scalar.activation(corrE, corr_ps, AT.Exp, accum_out=rsum)
                    rinv = sp.tile([128, 1], F32, tag="rinv", bufs=4)
                    nc.vector.reciprocal(rinv, rsum)
                    corrBN = ap_.tile([128, 128], R32, tag="corrBN", bufs=2)
                    nc.vector.tensor_scalar(corrBN, corrE, rinv, vgc, ALU.mult, ALU.mult)
                    nc.vector.tensor_add(corrBN, corrBN, gcI)
                    cT_ps = pt.tile([128, 128], R32, tag="t", name=f"cTps{l}_{c}")
                    nc.tensor.transpose(cT_ps, corrBN, ident)
                    nc.vector.tensor_copy(cT[:, c, :], cT_ps)

                # r2 rows-major and feature-major via two matmul sets
                vbcf = bcp.tile([128, 128], F32, tag="bc2", bufs=4, name=f"vbcf{l}")
                nc.gpsimd.dma_start(out=vbcf, in_=bc_ap(d["vbc"][l]))
                r2r = ap_.tile([128, BPC, DM], R32, tag="r2r")
                r2T = ap_.tile([128, KD, 2 * 128], R32, tag="r2T")
                for c in range(BPC):
                    rr_ps = pbig.tile([128, DM], F32, tag="big", name=f"rrps{l}_{c}")
                    mmr(rr_ps, cT[:, c, :], x_t[:, c, :], start=True, stop=True)
                    nc.scalar.activation(r2r[:, c, :], rr_ps, AT.Identity, bias=vbc)
                    for m in range(KD):
                        rt_ps = pt.tile([128, 128], F32, tag="t", name=f"rtps{l}_{c}_{m}")
                        mmr(rt_ps, x_t[:, c, m * 128:(m + 1) * 128],
                            cT[:, c, :], start=True, stop=True)
                        # feature-major r2T: BN beta is along the free (channel)
                        # axis here, so add it via a partition-broadcast tile
                        nc.vector.tensor_add(r2T[:, m, c * 128:(c + 1) * 128],
                                             rt_ps, vbcf)

                if phase == "corr":
                    x_t = r2r
                    break
                x_t = _ffn_ln(nc, tile, mybir, bass, tc, ap_, sp, bcp, ph, pbig,
                              r2T, r2r, vw1t, vb1, vw2t, vb2b, vglb, vblb, l, "v", epsc,
                              vsb, inv_sqc, srow_of)
                if phase == "vc0":
                    break

                # ============ Auto-attention block ============
                # xT feature-major
                xT = ap_.tile([128, KD, 2 * 128], R32, tag="xT")
                for c in range(BPC):
                    for m in range(KD):
                        tp = pt.tile([128, 128], R32, tag="t", name=f"xTps{l}_{c}_{m}")
                        nc.tensor.transpose(tp, x_t[:, c, m * 128:(m + 1) * 128], ident)
                        nc.vector.tensor_copy(xT[:, m, c * 128:(c + 1) * 128], tp)

                # u = x @ M1 + c1   (rows-major out)
                u_t = ap_.tile([128, BPC, DM], F32, tag="u")
                for c in range(BPC):
                    u_ps = pbig.tile([128, DM], F32, tag="big", name=f"ups{l}_{c}")
                    for k in range(KD):
                        mmr(u_ps, xT[:, k, c * 128:(c + 1) * 128],
                            m1[:, k, :], start=(k == 0), stop=(k == KD - 1))
                    nc.vector.tensor_add(u_t[:, c, :], u_ps, c1b)

                if phase == "u":
                    x_t = u_t
                    break

                # scores S[r,i] = <u, roll_i(x)> * DM^-0.5 ; softmax over i
                scl = float(DM ** -0.5)
                Sp_t = ap_.tile([128, BPC, NS], F32, tag="Sp")
                trash = ap_.tile([128, DM], F32, tag="trash", bufs=1)
                for c in range(BPC):
                    # NOTE: tensor_tensor_reduce wedges the device on this
                    # walrus/NRT build (NRT_EXEC_UNIT_UNRECOVERABLE); use
                    # scalar_tensor_tensor's accum_out instead.
                    Sa = sp.tile([128, NS], F32, tag="Sa", bufs=2)
                    Sb = sp.tile([128, NS], F32, tag="Sb", bufs=2)
                    nc.vector.memset(Sb[:, 0:1], 0.0)
                    for i in range(NS):
                        sh = P * i
                        if sh == 0:
                            nc.vector.scalar_tensor_tensor(
                                out=trash, in0=u_t[:, c, :], scalar=scl,
                                in1=x_t[:, c, :], op0=ALU.mult, op1=ALU.mult,
                                accum_out=Sa[:, i:i + 1])
                        else:
                            nc.vector.scalar_tensor_tensor(
                                out=trash[:, :DM - sh], in0=u_t[:, c, :DM - sh],
                                scalar=scl, in1=x_t[:, c, sh:],
                                op0=ALU.mult, op1=ALU.mult, accum_out=Sa[:, i:i + 1])
                            nc.vector.scalar_tensor_tensor(
                                out=trash[:, DM - sh:], in0=u_t[:, c, DM - sh:],
                                scalar=scl, in1=x_t[:, c, :sh],
                                op0=ALU.mult, op1=ALU.mult, accum_out=Sb[:, i:i + 1])
                    S = sp.tile([128, NS], F32, tag="S", bufs=2)
                    nc.vector.tensor_add(S, Sa, Sb)
                    Se = sp.tile([128, NS], F32, tag="Se", bufs=2)
                    ssum = sp.tile([128, 1], F32, tag="ssum", bufs=4)
                    nc.scalar.activation(Se, S, AT.Exp, accum_out=ssum)
                    sinv = sp.tile([128, 1], F32, tag="sinv", bufs=4)
                    nc.vector.reciprocal(sinv, ssum)
                    nc.vector.tensor_scalar(Sp_t[:, c, :], Se, sinv, None, ALU.mult)

                if phase == "sc":
                    x_t = ap_.tile([128, BPC, DM], F32, tag="scdump")
                    nc.vector.memset(x_t, 0.0)
                    for c in range(BPC):
                        nc.vector.tensor_copy(x_t[:, c, 0:NS], Sp_t[:, c, :])
                    break

                # vm = sum_i p_i roll_i(x) via diag matmuls accumulating in PSUM
                vm_t = ap_.tile([128, BPC, DM], R32, tag="vm")
                for c in range(BPC):
                    vm_ps = pbig.tile([128, DM], F32, tag="big", name=f"vmps{l}_{c}")
                    for i in range(NS):
                        dg = ap_.tile([128, 128], R32, tag="dg", bufs=3)
                        nc.vector.tensor_scalar_mul(dg, ident, Sp_t[:, c, i:i + 1])
                        sh = P * i
                        last = (i == NS - 1)
                        if sh == 0:
                            mmr(vm_ps, dg, x_t[:, c, :], start=True, stop=False)
                        else:
                            mmr(vm_ps[:, :DM - sh], dg, x_t[:, c, sh:],
                                start=False, stop=False)
                            mmr(vm_ps[:, DM - sh:], dg, x_t[:, c, :sh],
                                start=False, stop=last)
                    nc.scalar.activation(vm_t[:, c, :], vm_ps, AT.Identity)

                if phase == "vm":
                    x_t = vm_t
                    break

                # vmT feature-major
                vmT = ap_.tile([128, KD, 2 * 128], R32, tag="vmT")
                for c in range(BPC):
                    for m in range(KD):
                        tp2 = pt.tile([128, 128], R32, tag="t", name=f"vmTps{l}_{c}_{m}")
                        nc.tensor.transpose(tp2, vm_t[:, c, m * 128:(m + 1) * 128], ident)
                        nc.scalar.activation(vmT[:, m, c * 128:(c + 1) * 128], tp2, AT.Identity)

                # attn out rows-major: o = vm @ M2 + c2 ; r1 = BN(o + x)
                r1r = ap_.tile([128, BPC, DM], R32, tag="r1r")
                for c in range(BPC):
                    o_ps = pbig.tile([128, DM], F32, tag="big", name=f"ops{l}_{c}")
                    for k in range(KD):
                        mmr(o_ps, vmT[:, k, c * 128:(c + 1) * 128],
                            m2[:, k, :], start=(k == 0), stop=(k == KD - 1))
                    t1 = ap_.tile([128, DM], F32, tag="t1", bufs=2)
                    nc.vector.tensor_add(t1, o_ps, x_t[:, c, :])
                    nc.vector.tensor_add(t1, t1, c2b)
                    nc.scalar.activation(r1r[:, c, :], t1, AT.Identity, bias=abc, scale=agc)

                if phase == "attn":
                    x_t = r1r
                    break

                # r1T feature-major
                r1T = ap_.tile([128, KD, 2 * 128], R32, tag="r1T")
                for c in range(BPC):
                    for m in range(KD):
                        tp3 = pt.tile([128, 128], R32, tag="t", name=f"r1Tps{l}_{c}_{m}")
                        nc.tensor.transpose(tp3, r1r[:, c, m * 128:(m + 1) * 128], ident)
                        nc.scalar.activation(r1T[:, m, c * 128:(c + 1) * 128], tp3, AT.Identity)

                x_t = _ffn_ln(nc, tile, mybir, bass, tc, ap_, sp, bcp, ph, pbig,
                              r1T, r1r, aw1t, ab1, aw2t, ab2b, aglb, ablb, l, "a", epsc,
                              asb, inv_sqc, srow_of)

            # ---------------- store ----------------
            for c in range(BPC):
                nc.sync.dma_start(out=out_d.ap()[c], in_=x_t[:, c, :].bitcast(F32))


def _ffn_ln(nc, tile, mybir, bass, tc, ap_, sp, bcp, ph, pbig,
            rT, rrows, w1t, b1, w2t, b2b, glb, blb, l, pfx, epsc,
            sumb, inv_sqc, srow_of):
    """h = gelu(r @ W1.T + b1); y = h @ W2.T + b2; x = LN(y + r) * g + b.

    Also emits (for the "a" blocks feeding the next varcor) the row-sum of
    the next x via <xn, g> + sum(b) so the correlation chain never waits on
    the gamma/beta affine (which runs on GpSimd off the critical path)."""
    F32 = mybir.dt.float32
    R32 = mybir.dt.float32r
    AT = mybir.ActivationFunctionType
    ALU = mybir.AluOpType
    AX = mybir.AxisListType

    hT = ap_.tile([128, KH, 2 * 128], R32, tag="hT", bufs=1, name=f"hT{pfx}{l}")
    for mh2 in range(KH // 2):
        h_ps = ph.tile([128, 2, 128 * 2], F32, tag="h", name=f"hps{pfx}{l}_{mh2}")
        for half in range(2):
            mh = mh2 * 2 + half
            for k in range(KD):
                nc.tensor.matmul(h_ps[:, half, :], w1t[:, k, mh * 128:(mh + 1) * 128],
                                 rT[:, k, :], start=(k == 0), stop=(k == KD - 1))
            nc.scalar.activation(hT[:, mh, :], h_ps[:, half, :], AT.Gelu,
                                 bias=b1[:, mh:mh + 1])

    x_new = ap_.tile([128, BPC, DM], R32, tag="x", bufs=3, name=f"x{pfx}{l}")
    for c in range(BPC):
        y_ps = pbig.tile([128, DM], F32, tag="big", name=f"yps{pfx}{l}_{c}")
        for k in range(KH):
            nc.tensor.matmul(y_ps, hT[:, k, c * 128:(c + 1) * 128],
                             w2t[:, k, :], start=(k == 0), stop=(k == KH - 1))
        rb = ap_.tile([128, DM], F32, tag="rb", bufs=2, name=f"rb{pfx}{l}_{c}")
        nc.vector.tensor_add(rb, rrows[:, c, :], b2b)
        z = ap_.tile([128, DM], F32, tag="z", bufs=2, name=f"z{pfx}{l}_{c}")
        nc.vector.tensor_add(z, y_ps, rb)
        st6 = sp.tile([128, 6], F32, tag="st6", bufs=4)
        nc.vector.bn_stats(out=st6, in_=z)
        mv = sp.tile([128, 2], F32, tag="mv", bufs=4)
        nc.vector.bn_aggr(out=mv, in_=st6)
        std = sp.tile([128, 1], F32, tag="std", bufs=4)
        nc.scalar.activation(std, mv[:, 1:2], AT.Sqrt, bias=epsc)
        rstd = sp.tile([128, 1], F32, tag="rstd", bufs=4)
        nc.vector.reciprocal(rstd, std)
        nb = sp.tile([128, 1], F32, tag="nb", bufs=4)
        nc.vector.tensor_scalar_mul(nb, mv[:, 0:1], -1.0)
        xn = ap_.tile([128, DM], F32, tag="xn", bufs=2, name=f"xn{pfx}{l}_{c}")
        nc.vector.tensor_scalar(xn, z, nb, rstd, ALU.add, ALU.mult)
        if pfx == "a" and l < L - 1:
            # next-layer corr row-sum: <xn*rstd-normalized x, g> + sum(b)
            trash2 = ap_.tile([128, DM], F32, tag="trash", bufs=1,
                              name=f"tr2{pfx}{l}_{c}")
            sraw = sp.tile([128, 1], F32, tag="sraw", bufs=4)
            nc.vector.scalar_tensor_tensor(
                out=trash2, in0=xn, scalar=1.0, in1=glb,
                op0=ALU.mult, op1=ALU.mult, accum_out=sraw)
            srow = sp.tile([128, 1], F32, tag="srow", bufs=4, name=f"srow{pfx}{l}_{c}")
            nc.scalar.activation(srow, sraw, AT.Identity, bias=sumb)
            srow_of[c] = srow
        # affine (the next-layer corr chain does not wait on it: srow above)
        nc.vector.tensor_mul(x_new[:, c, :], xn, glb)
        nc.vector.tensor_add(x_new[:, c, :], x_new[:, c, :], blb)
    return x_new


# ======================================================================
# host side
# ======================================================================

_COMPILED = {}


def _compile():
    if "nc" in _COMPILED:
        return _COMPILED["nc"]
    import concourse.bass as bass
    import concourse.bacc as bacc
    import concourse.tile as tile
    from concourse import mybir
    nc = bacc.Bacc("TRN2", target_bir_lowering=False, debug=False, num_devices=NC_)
    _build(nc, tile, mybir, bass)
    nc.compile()
    _COMPILED["nc"] = nc
    return nc


def _host_prep(inputs):
    f = lambda k: np.asarray(inputs[k], np.float32)
    ld_w = f("ld_w").reshape(KS).astype(np.float64)
    # conv matrix with replicate padding, R = I - S
    S = np.zeros((T, T), np.float64)
    idx = np.clip(np.arange(T)[:, None] + np.arange(KS)[None, :] - KS // 2, 0, T - 1)
    for k in range(KS):
        np.add.at(S, (np.arange(T), idx[:, k]), ld_w[k])
    Rm = np.eye(T) - S
    emb_W = f("emb_W").astype(np.float64)
    memb = (Rm.T @ emb_W.T).astype(np.float32)              # (T, DM)
    wpos = (f("W_pos") + f("emb_b")[None, :]
            - float(f("ld_b")[0]) * emb_W.sum(1).astype(np.float32)[None, :])

    g = {"memb": np.ascontiguousarray(memb.reshape(KD, 128, DM).transpose(1, 0, 2)),
         "wpos": np.ascontiguousarray(wpos.astype(np.float32)),
         "ident": np.eye(128, dtype=np.float32)}

    s1 = np.float32(1.0 / np.sqrt(1.0 + EPS))
    def stack(fn):
        return np.ascontiguousarray(np.stack([fn(l) for l in range(L)]).astype(np.float32))

    def shuf(a):
        # (k*128, n) -> (128, k, n): SBUF layout with contiguous per-partition rows
        kn, n = a.shape
        return a.reshape(kn // 128, 128, n).transpose(1, 0, 2)

    g["vw1t"] = stack(lambda l: shuf(f("vc_W1")[l].T))
    g["vb1"] = stack(lambda l: f("vc_b1")[l].reshape(KH, 128).T)
    g["vw2t"] = stack(lambda l: shuf(f("vc_W2")[l].T))
    g["vb2"] = stack(lambda l: f("vc_b2")[l])
    g["aw1t"] = stack(lambda l: shuf(f("aa_W1")[l].T))
    g["ab1"] = stack(lambda l: f("aa_b1")[l].reshape(KH, 128).T)
    g["aw2t"] = stack(lambda l: shuf(f("aa_W2")[l].T))
    g["ab2"] = stack(lambda l: f("aa_b2")[l])
    g["m1"] = stack(lambda l: shuf(f("aa_Wq")[l].astype(np.float64).T @ f("aa_Wk")[l].astype(np.float64)))
    g["c1"] = stack(lambda l: f("aa_bq")[l].astype(np.float64) @ f("aa_Wk")[l].astype(np.float64))
    g["m2"] = stack(lambda l: shuf((f("aa_Wo")[l].astype(np.float64) @ f("aa_Wv")[l].astype(np.float64)).T))
    g["c2"] = stack(lambda l: f("aa_bv")[l].astype(np.float64) @ f("aa_Wo")[l].astype(np.float64).T
                    + f("aa_bo")[l].astype(np.float64))
    g["vsb"] = stack(lambda l: f("vc_ln_b")[l].sum(keepdims=True))
    g["asb"] = stack(lambda l: f("aa_ln_b")[l].sum(keepdims=True))
    g["vgc"] = stack(lambda l: f("vc_bn_g")[l] * s1)
    g["vbc"] = stack(lambda l: f("vc_bn_b")[l])
    g["vgl"] = stack(lambda l: f("vc_ln_g")[l])
    g["vbl"] = stack(lambda l: f("vc_ln_b")[l])
    g["agc"] = stack(lambda l: f("aa_bn_g")[l] * s1)
    g["abc"] = stack(lambda l: f("aa_bn_b")[l])
    g["agl"] = stack(lambda l: f("aa_ln_g")[l])
    g["abl"] = stack(lambda l: f("aa_ln_b")[l])
    return g


def kernel(**inputs):
    from concourse.bass_utils import run_bass_kernel_spmd
    nc = _compile()
    g = _host_prep(inputs)
    inp = np.asarray(inputs["inp"], np.float32)
    in_maps = []
    for core in range(NC_):
        m = dict(g)
        sl = inp[core * BPC:(core + 1) * BPC]          # (BPC, T, C)
        m["xin"] = np.ascontiguousarray(
            sl.reshape(BPC, KD, 128, C).transpose(2, 0, 1, 3))
        in_maps.append(m)
    res = run_bass_kernel_spmd(nc, in_maps, core_ids=list(range(NC_)))
    if res.exec_time_ns is not None:
        kernel.last_exec_time_ns = res.exec_time_ns
    out = np.concatenate([res.results[k]["out"] for k in range(NC_)], axis=0)
    return out


kernel.last_exec_time_ns = None


# revision 28
# speedup vs baseline: 1.3288x; 1.1682x over previous
"""CAWformer forward on 8 TRN2 NeuronCores — data parallel over batch.

Math notes (all exact algebraic rewrites of the reference):
  * irfft(xf_i * conj(xf_j)).mean(-1) == s_i * s_j / DM with s = x.sum(-1),
    so the FFT cross-correlation attention is softmax(outer(s, s)/c) @ x.
  * The 8-shift auto-attention: scores_i = <q@Wk, roll_i(x)> (+const that
    cancels in softmax); out = (sum_i p_i roll_i(x)) @ Wv.T @ Wo.T + const.
  * The depthwise smoothing conv is a (T,T) band matrix S; residual embed
    folds to inp[b].T @ (R.T @ emb_W.T) with R = I - S.
Per-core layout: rows = 2 batches x 128 channels, processed as two
128-partition chunks. Activations live rows-major (channels on partitions,
d_model on free axis); matmul operands are built feature-major via PE
transposes / algebraic tricks.
"""

import os
import numpy as np

B, T, C, DM, L, P, KS = 16, 512, 128, 512, 3, 64, 25
EPS = 1e-5
NS = DM // P           # 8 circular shifts
NC_ = 8                # cores
BPC = B // NC_         # batches per core = 2
H = 2 * DM             # FFN hidden = 1024
KD = DM // 128         # 4 k-tiles over d_model
KH = H // 128          # 8 k-tiles over hidden

F32 = None  # set lazily (mybir.dt.float32)


def _build(nc, tile, mybir, bass):
    F32 = mybir.dt.float32
    R32 = mybir.dt.float32r
    AT = mybir.ActivationFunctionType
    ALU = mybir.AluOpType
    AX = mybir.AxisListType

    def mmr(out, lhsT, rhs, start, stop):
        # float32r: single-pass PE matmul (4x the fp32 rate at N>=256),
        # fp32 memory format, ~tf32 multiply precision, fp32 accumulate.
        # Operand tiles are allocated float32r end-to-end because the BIR
        # verifier requires fp32r matmult inputs be *produced* as fp32r.
        nc.tensor.matmul(out, lhsT, rhs, start=start, stop=stop)

    # ---------------- DRAM I/O ----------------
    d = {}
    def din(name, shape, dt_=None):
        d[name] = nc.dram_tensor(name, list(shape), dt_ or F32, kind="ExternalInput")
        return d[name]

    # weight layouts are pre-shuffled on host to (128, k, n) so every DMA
    # is 128 partitions x contiguous-per-partition (full-bandwidth descriptors)
    din("xin", (128, BPC, KD, C), R32)
    din("memb", (128, KD, DM), R32)
    din("wpos", (C, DM))
    din("ident", (128, 128), R32)
    din("vw1t", (L, 128, KD, H), R32); din("vb1", (L, 128, KH)); din("vw2t", (L, 128, KH, DM), R32); din("vb2", (L, DM))
    din("aw1t", (L, 128, KD, H), R32); din("ab1", (L, 128, KH)); din("aw2t", (L, 128, KH, DM), R32); din("ab2", (L, DM))
    din("m1", (L, 128, KD, DM), R32); din("c1", (L, DM))
    din("m2", (L, 128, KD, DM), R32); din("c2", (L, DM))
    din("vsb", (L, 1)); din("asb", (L, 1))
    din("vgc", (L, C)); din("vbc", (L, C)); din("vgl", (L, DM)); din("vbl", (L, DM))
    din("agc", (L, C)); din("abc", (L, C)); din("agl", (L, DM)); din("abl", (L, DM))
    out_d = nc.dram_tensor("out", [BPC, C, DM], F32, kind="ExternalOutput")

    def bc_ap(src, parts=128):
        # broadcast a DRAM vector AP across partitions
        return bass.AP(tensor=src.tensor, offset=src.offset,
                       ap=[[0, parts]] + [list(x) for x in src.ap])

    def col_ap(src):
        # DRAM vector (n,) -> (n,1) partition-major AP
        return bass.AP(tensor=src.tensor, offset=src.offset,
                       ap=[list(src.ap[0]), [0, 1]])

    with tile.TileContext(nc) as tc:
        import contextlib
        ctx = contextlib.ExitStack()
        with ctx:
            wp = ctx.enter_context(tc.tile_pool(name="wp", bufs=1))
            ap_ = ctx.enter_context(tc.tile_pool(name="ap", bufs=1))
            bcp = ctx.enter_context(tc.tile_pool(name="bcp", bufs=8))
            sp = ctx.enter_context(tc.tile_pool(name="sp", bufs=8))
            cp = ctx.enter_context(tc.tile_pool(name="cp", bufs=1))
            pbig = ctx.enter_context(tc.tile_pool(name="pbig", bufs=3, space="PSUM"))
            ph = ctx.enter_context(tc.tile_pool(name="ph", bufs=2, space="PSUM"))
            pt = ctx.enter_context(tc.tile_pool(name="pt", bufs=2, space="PSUM"))

            # ---------------- constants ----------------
            ident = cp.tile([128, 128], R32)
            nc.sync.dma_start(out=ident, in_=d["ident"].ap())
            epsc = cp.tile([128, 1], F32)
            nc.vector.memset(epsc, EPS)
            memb_sb = cp.tile([128, KD, DM], R32)
            nc.sync.dma_start(out=memb_sb, in_=d["memb"].ap())
            wpos_sb = cp.tile([128, DM], F32)
            nc.sync.dma_start(out=wpos_sb, in_=d["wpos"].ap())
            xin_sb = cp.tile([128, BPC, KD, C], R32)
            nc.sync.dma_start(out=xin_sb, in_=d["xin"].ap())

            # ---------------- embed:  x[c] = xin[c].T @ memb + wpos ----------------
            x_t = ap_.tile([128, BPC, DM], R32, tag="x", bufs=3)
            for c in range(BPC):
                x_ps = pbig.tile([128, DM], F32, tag="big")
                for k in range(KD):
                    mmr(x_ps, xin_sb[:, c, k, :], memb_sb[:, k, :],
                        start=(k == 0), stop=(k == KD - 1))
                nc.vector.tensor_add(x_t[:, c, :], x_ps, wpos_sb)

            inv_sqc = float(1.0 / np.sqrt(DM * np.sqrt(DM)))
            phase = os.environ.get("KPHASE", "full")
            srow_of = {}

            # ---------------- layers ----------------
            for l in range(L if phase == "full" else 1):
                if phase == "emb":
                    break
                # ---- layer weight loads ----
                vw1t = wp.tile([128, KD, H], R32, tag="vw1t")
                nc.sync.dma_start(out=vw1t, in_=d["vw1t"][l])
                vb1 = sp.tile([128, KH], F32, tag="vb1")
                nc.sync.dma_start(out=vb1, in_=d["vb1"][l])
                vw2t = wp.tile([128, KH, DM], R32, tag="vw2t")
                nc.sync.dma_start(out=vw2t, in_=d["vw2t"][l])
                aw1t = wp.tile([128, KD, H], R32, tag="aw1t")
                nc.sync.dma_start(out=aw1t, in_=d["aw1t"][l])
                ab1 = sp.tile([128, KH], F32, tag="ab1")
                nc.sync.dma_start(out=ab1, in_=d["ab1"][l])
                aw2t = wp.tile([128, KH, DM], R32, tag="aw2t")
                nc.sync.dma_start(out=aw2t, in_=d["aw2t"][l])
                m1 = wp.tile([128, KD, DM], R32, tag="m1")
                nc.sync.dma_start(out=m1, in_=d["m1"][l])
                m2 = wp.tile([128, KD, DM], R32, tag="m2")
                nc.sync.dma_start(out=m2, in_=d["m2"][l])

                vgc = sp.tile([128, 1], F32, tag="vgc")
                nc.gpsimd.dma_start(out=vgc, in_=col_ap(d["vgc"][l]))
                agc = sp.tile([128, 1], F32, tag="agc")
                nc.gpsimd.dma_start(out=agc, in_=col_ap(d["agc"][l]))
                vbc = sp.tile([128, 1], F32, tag="vbc")
                nc.gpsimd.dma_start(out=vbc, in_=col_ap(d["vbc"][l]))
                abc = sp.tile([128, 1], F32, tag="abc")
                nc.gpsimd.dma_start(out=abc, in_=col_ap(d["abc"][l]))
                vbcf = bcp.tile([128, 128], F32, tag="bc2", bufs=4, name=f"vbcf{l}")
                nc.gpsimd.dma_start(out=vbcf, in_=bc_ap(d["vbc"][l]))

                def bcast(name):
                    t = bcp.tile([128, DM], F32, tag="bc", name=f"{name}_bc{l}")
                    nc.gpsimd.dma_start(out=t, in_=bc_ap(d[name][l]))
                    return t
                c1b = bcast("c1"); c2b = bcast("c2")
                vb2b = bcast("vb2"); ab2b = bcast("ab2")
                vglb = bcast("vgl"); vblb = bcast("vbl")
                aglb = bcast("agl"); ablb = bcast("abl")

                # gcI = diag(gc_vc) as dense tile for the "+I" residual fold
                gcI = sp.tile([128, 128], F32, tag="gcI", bufs=2)
                nc.vector.tensor_scalar_mul(gcI, ident, vgc)
                vsb = sp.tile([128, 1], F32, tag="vsb")
                nc.gpsimd.dma_start(out=vsb, in_=bc_ap(d["vsb"][l]))
                asb = sp.tile([128, 1], F32, tag="asb")
                nc.gpsimd.dma_start(out=asb, in_=bc_ap(d["asb"][l]))

                # ============ VarCor block ============
                # s = rowsum(x) * 1/sqrt(DM*sqrt(DM)) (split sqrt per side)
                cT = ap_.tile([128, BPC, 128], R32, tag="cT")
                for c in range(BPC):
                    if c in srow_of:
                        srow = srow_of[c]
                    else:
                        srow = sp.tile([128, 1], F32, tag="srow", bufs=4)
                        nc.vector.tensor_reduce(srow, x_t[:, c, :], AX.X, ALU.add)
                    s2 = sp.tile([128, 1], R32, tag="s2", bufs=4)
                    nc.scalar.mul(s2, srow, inv_sqc)
                    sT_ps = pbig.tile([1, 128], R32, tag="big", name=f"sTps{l}_{c}")
                    nc.tensor.transpose(sT_ps, s2, ident)
                    sT = sp.tile([1, 128], R32, tag="sT", bufs=4)
                    nc.scalar.activation(sT, sT_ps, AT.Identity)
                    corr_ps = pbig.tile([128, 128], F32, tag="big", name=f"corrps{l}_{c}")
                    mmr(corr_ps, sT, sT, start=True, stop=True)
                    # softmax over free axis (values are O(1): skip max-sub)
                    # + BN row-scale + +I fold
                    corrE = ap_.tile([128, 128], F32, tag="corrE", bufs=2)
                    rsum = sp.tile([128, 1], F32, tag="rsum", bufs=4)
                    nc.scalar.activation(corrE, corr_ps, AT.Exp, accum_out=rsum)
                    rinv = sp.tile([128, 1], F32, tag="rinv", bufs=4)
                    nc.vector.reciprocal(rinv, rsum)
                    corrBN = ap_.tile([128, 128], R32, tag="corrBN", bufs=2)
                    nc.vector.tensor_scalar(corrBN, corrE, rinv, vgc, ALU.mult, ALU.mult)
                    nc.vector.tensor_add(corrBN, corrBN, gcI)
                    cT_ps = pt.tile([128, 128], R32, tag="t", name=f"cTps{l}_{c}")
                    nc.tensor.transpose(cT_ps, corrBN, ident)
                    nc.vector.tensor_copy(cT[:, c, :], cT_ps)

                # r2 rows-major and feature-major via two matmul sets
                r2r = ap_.tile([128, BPC, DM], R32, tag="r2r")
                r2T = ap_.tile([128, KD, 2 * 128], R32, tag="r2T")
                for c in range(BPC):
                    rr_ps = pbig.tile([128, DM], F32, tag="big", name=f"rrps{l}_{c}")
                    mmr(rr_ps, cT[:, c, :], x_t[:, c, :], start=True, stop=True)
                    nc.scalar.activation(r2r[:, c, :], rr_ps, AT.Identity, bias=vbc)
                    for m in range(KD):
                        rt_ps = pt.tile([128, 128], F32, tag="t", name=f"rtps{l}_{c}_{m}")
                        mmr(rt_ps, x_t[:, c, m * 128:(m + 1) * 128],
                            cT[:, c, :], start=True, stop=True)
                        # feature-major r2T: BN beta is along the free (channel)
                        # axis here, so add it via a partition-broadcast tile
                        nc.vector.tensor_add(r2T[:, m, c * 128:(c + 1) * 128],
                                             rt_ps, vbcf)

                if phase == "corr":
                    x_t = r2r
                    break
                x_t = _ffn_ln(nc, tile, mybir, bass, tc, ap_, sp, bcp, ph, pbig,
                              r2T, r2r, vw1t, vb1, vw2t, vb2b, vglb, vblb, l, "v", epsc,
                              vsb, inv_sqc, srow_of)
                if phase == "vc0":
                    break

                # ============ Auto-attention block ============
                # xT feature-major
                xT = ap_.tile([128, KD, 2 * 128], R32, tag="xT")
                for c in range(BPC):
                    for m in range(KD):
                        tp = pt.tile([128, 128], R32, tag="t", name=f"xTps{l}_{c}_{m}")
                        nc.tensor.transpose(tp, x_t[:, c, m * 128:(m + 1) * 128], ident)
                        nc.vector.tensor_copy(xT[:, m, c * 128:(c + 1) * 128], tp)

                # u = x @ M1 + c1   (rows-major out)
                u_t = ap_.tile([128, BPC, DM], F32, tag="u")
                for c in range(BPC):
                    u_ps = pbig.tile([128, DM], F32, tag="big", name=f"ups{l}_{c}")
                    for k in range(KD):
                        mmr(u_ps, xT[:, k, c * 128:(c + 1) * 128],
                            m1[:, k, :], start=(k == 0), stop=(k == KD - 1))
                    nc.vector.tensor_add(u_t[:, c, :], u_ps, c1b)

                if phase == "u":
                    x_t = u_t
                    break

                # scores S[r,i] = <u, roll_i(x)> * DM^-0.5 ; softmax over i
                scl = float(DM ** -0.5)
                Sp_t = ap_.tile([128, BPC, NS], F32, tag="Sp")
                trash = ap_.tile([128, DM], F32, tag="trash", bufs=1)
                for c in range(BPC):
                    # NOTE: tensor_tensor_reduce wedges the device on this
                    # walrus/NRT build (NRT_EXEC_UNIT_UNRECOVERABLE); use
                    # scalar_tensor_tensor's accum_out instead.
                    Sa = sp.tile([128, NS], F32, tag="Sa", bufs=2)
                    Sb = sp.tile([128, NS], F32, tag="Sb", bufs=2)
                    nc.vector.memset(Sb[:, 0:1], 0.0)
                    for i in range(NS):
                        sh = P * i
                        if sh == 0:
                            nc.vector.scalar_tensor_tensor(
                                out=trash, in0=u_t[:, c, :], scalar=scl,
                                in1=x_t[:, c, :], op0=ALU.mult, op1=ALU.mult,
                                accum_out=Sa[:, i:i + 1])
                        else:
                            nc.vector.scalar_tensor_tensor(
                                out=trash[:, :DM - sh], in0=u_t[:, c, :DM - sh],
                                scalar=scl, in1=x_t[:, c, sh:],
                                op0=ALU.mult, op1=ALU.mult, accum_out=Sa[:, i:i + 1])
                            nc.vector.scalar_tensor_tensor(
                                out=trash[:, DM - sh:], in0=u_t[:, c, DM - sh:],
                                scalar=scl, in1=x_t[:, c, :sh],
                                op0=ALU.mult, op1=ALU.mult, accum_out=Sb[:, i:i + 1])
                    S = sp.tile([128, NS], F32, tag="S", bufs=2)
                    nc.vector.tensor_add(S, Sa, Sb)
                    Se = sp.tile([128, NS], F32, tag="Se", bufs=2)
                    ssum = sp.tile([128, 1], F32, tag="ssum", bufs=4)
                    nc.scalar.activation(Se, S, AT.Exp, accum_out=ssum)
                    sinv = sp.tile([128, 1], F32, tag="sinv", bufs=4)
                    nc.vector.reciprocal(sinv, ssum)
                    nc.vector.tensor_scalar(Sp_t[:, c, :], Se, sinv, None, ALU.mult)

                if phase == "sc":
                    x_t = ap_.tile([128, BPC, DM], F32, tag="scdump")
                    nc.vector.memset(x_t, 0.0)
                    for c in range(BPC):
                        nc.vector.tensor_copy(x_t[:, c, 0:NS], Sp_t[:, c, :])
                    break

                # vm = sum_i p_i roll_i(x) via diag matmuls accumulating in PSUM
                vm_t = ap_.tile([128, BPC, DM], R32, tag="vm")
                for c in range(BPC):
                    vm_ps = pbig.tile([128, DM], F32, tag="big", name=f"vmps{l}_{c}")
                    for i in range(NS):
                        dg = ap_.tile([128, 128], R32, tag="dg", bufs=3)
                        nc.vector.tensor_scalar_mul(dg, ident, Sp_t[:, c, i:i + 1])
                        sh = P * i
                        last = (i == NS - 1)
                        if sh == 0:
                            mmr(vm_ps, dg, x_t[:, c, :], start=True, stop=False)
                        else:
                            mmr(vm_ps[:, :DM - sh], dg, x_t[:, c, sh:],
                                start=False, stop=False)
                            mmr(vm_ps[:, DM - sh:], dg, x_t[:, c, :sh],
                                start=False, stop=last)
                    nc.scalar.activation(vm_t[:, c, :], vm_ps, AT.Identity)

                if phase == "vm":
                    x_t = vm_t
                    break

                # vmT feature-major
                vmT = ap_.tile([128, KD, 2 * 128], R32, tag="vmT")
                for c in range(BPC):
                    for m in range(KD):
                        tp2 = pt.tile([128, 128], R32, tag="t", name=f"vmTps{l}_{c}_{m}")
                        nc.tensor.transpose(tp2, vm_t[:, c, m * 128:(m + 1) * 128], ident)
                        nc.scalar.activation(vmT[:, m, c * 128:(c + 1) * 128], tp2, AT.Identity)

                # attn out rows-major: o = vm @ M2 + c2 ; r1 = BN(o + x)
                r1r = ap_.tile([128, BPC, DM], R32, tag="r1r")
                for c in range(BPC):
                    o_ps = pbig.tile([128, DM], F32, tag="big", name=f"ops{l}_{c}")
                    for k in range(KD):
                        mmr(o_ps, vmT[:, k, c * 128:(c + 1) * 128],
                            m2[:, k, :], start=(k == 0), stop=(k == KD - 1))
                    t1 = ap_.tile([128, DM], F32, tag="t1", bufs=2)
                    nc.vector.tensor_add(t1, o_ps, x_t[:, c, :])
                    nc.vector.tensor_add(t1, t1, c2b)
                    nc.scalar.activation(r1r[:, c, :], t1, AT.Identity, bias=abc, scale=agc)

                if phase == "attn":
                    x_t = r1r
                    break

                # r1T feature-major
                r1T = ap_.tile([128, KD, 2 * 128], R32, tag="r1T")
                for c in range(BPC):
                    for m in range(KD):
                        tp3 = pt.tile([128, 128], R32, tag="t", name=f"r1Tps{l}_{c}_{m}")
                        nc.tensor.transpose(tp3, r1r[:, c, m * 128:(m + 1) * 128], ident)
                        nc.scalar.activation(r1T[:, m, c * 128:(c + 1) * 128], tp3, AT.Identity)

                x_t = _ffn_ln(nc, tile, mybir, bass, tc, ap_, sp, bcp, ph, pbig,
                              r1T, r1r, aw1t, ab1, aw2t, ab2b, aglb, ablb, l, "a", epsc,
                              asb, inv_sqc, srow_of)

            # ---------------- store ----------------
            for c in range(BPC):
                nc.sync.dma_start(out=out_d.ap()[c], in_=x_t[:, c, :].bitcast(F32))


def _ffn_ln(nc, tile, mybir, bass, tc, ap_, sp, bcp, ph, pbig,
            rT, rrows, w1t, b1, w2t, b2b, glb, blb, l, pfx, epsc,
            sumb, inv_sqc, srow_of):
    """h = gelu(r @ W1.T + b1); y = h @ W2.T + b2; x = LN(y + r) * g + b.

    Also emits (for the "a" blocks feeding the next varcor) the row-sum of
    the next x via <xn, g> + sum(b) so the correlation chain never waits on
    the gamma/beta affine (which runs on GpSimd off the critical path)."""
    F32 = mybir.dt.float32
    R32 = mybir.dt.float32r
    AT = mybir.ActivationFunctionType
    ALU = mybir.AluOpType
    AX = mybir.AxisListType

    hT = ap_.tile([128, KH, 2 * 128], R32, tag="hT", bufs=2, name=f"hT{pfx}{l}")
    for mh2 in range(KH // 2):
        h_ps = ph.tile([128, 2, 128 * 2], F32, tag="h", name=f"hps{pfx}{l}_{mh2}")
        for half in range(2):
            mh = mh2 * 2 + half
            for k in range(KD):
                nc.tensor.matmul(h_ps[:, half, :], w1t[:, k, mh * 128:(mh + 1) * 128],
                                 rT[:, k, :], start=(k == 0), stop=(k == KD - 1))
            nc.scalar.activation(hT[:, mh, :], h_ps[:, half, :], AT.Gelu,
                                 bias=b1[:, mh:mh + 1])
    # prewarm the Sqrt activation table while FFN2 runs so the LN-critical
    # Sqrt below hits a warm table (ACT table switches cost ~1.3us)
    stdw = sp.tile([128, 1], F32, tag="std", bufs=4, name=f"stdw{pfx}{l}")
    nc.scalar.activation(stdw, epsc, AT.Sqrt, bias=epsc)

    x_new = ap_.tile([128, BPC, DM], R32, tag="x", bufs=3, name=f"x{pfx}{l}")
    for c in range(BPC):
        y_ps = pbig.tile([128, DM], F32, tag="big", name=f"yps{pfx}{l}_{c}")
        for k in range(KH):
            nc.tensor.matmul(y_ps, hT[:, k, c * 128:(c + 1) * 128],
                             w2t[:, k, :], start=(k == 0), stop=(k == KH - 1))
        rb = ap_.tile([128, DM], F32, tag="rb", bufs=2, name=f"rb{pfx}{l}_{c}")
        nc.vector.tensor_add(rb, rrows[:, c, :], b2b)
        z = ap_.tile([128, DM], F32, tag="z", bufs=2, name=f"z{pfx}{l}_{c}")
        nc.vector.tensor_add(z, y_ps, rb)
        st6 = sp.tile([128, 6], F32, tag="st6", bufs=4)
        nc.vector.bn_stats(out=st6, in_=z)
        mv = sp.tile([128, 2], F32, tag="mv", bufs=4)
        nc.vector.bn_aggr(out=mv, in_=st6)
        std = sp.tile([128, 1], F32, tag="std", bufs=4)
        nc.scalar.activation(std, mv[:, 1:2], AT.Sqrt, bias=epsc)
        rstd = sp.tile([128, 1], F32, tag="rstd", bufs=4)
        nc.vector.reciprocal(rstd, std)
        nb = sp.tile([128, 1], F32, tag="nb", bufs=4)
        nc.vector.tensor_scalar_mul(nb, mv[:, 0:1], -1.0)
        xn = ap_.tile([128, DM], F32, tag="xn", bufs=2, name=f"xn{pfx}{l}_{c}")
        nc.vector.tensor_scalar(xn, z, nb, rstd, ALU.add, ALU.mult)
        if pfx == "a" and l < L - 1:
            # next-layer corr row-sum: <xn*rstd-normalized x, g> + sum(b)
            trash2 = ap_.tile([128, DM], F32, tag="trash", bufs=1,
                              name=f"tr2{pfx}{l}_{c}")
            sraw = sp.tile([128, 1], F32, tag="sraw", bufs=4)
            nc.vector.scalar_tensor_tensor(
                out=trash2, in0=xn, scalar=1.0, in1=glb,
                op0=ALU.mult, op1=ALU.mult, accum_out=sraw)
            srow = sp.tile([128, 1], F32, tag="srow", bufs=4, name=f"srow{pfx}{l}_{c}")
            nc.scalar.activation(srow, sraw, AT.Identity, bias=sumb)
            srow_of[c] = srow
        # affine (the next-layer corr chain does not wait on it: srow above)
        nc.vector.tensor_mul(x_new[:, c, :], xn, glb)
        nc.vector.tensor_add(x_new[:, c, :], x_new[:, c, :], blb)
    return x_new


# ======================================================================
# host side
# ======================================================================

_COMPILED = {}


def _compile():
    if "nc" in _COMPILED:
        return _COMPILED["nc"]
    import concourse.bass as bass
    import concourse.bacc as bacc
    import concourse.tile as tile
    from concourse import mybir
    nc = bacc.Bacc("TRN2", target_bir_lowering=False, debug=False, num_devices=NC_)
    _build(nc, tile, mybir, bass)
    nc.compile()
    _COMPILED["nc"] = nc
    return nc


def _host_prep(inputs):
    f = lambda k: np.asarray(inputs[k], np.float32)
    ld_w = f("ld_w").reshape(KS).astype(np.float64)
    # conv matrix with replicate padding, R = I - S
    S = np.zeros((T, T), np.float64)
    idx = np.clip(np.arange(T)[:, None] + np.arange(KS)[None, :] - KS // 2, 0, T - 1)
    for k in range(KS):
        np.add.at(S, (np.arange(T), idx[:, k]), ld_w[k])
    Rm = np.eye(T) - S
    emb_W = f("emb_W").astype(np.float64)
    memb = (Rm.T @ emb_W.T).astype(np.float32)              # (T, DM)
    wpos = (f("W_pos") + f("emb_b")[None, :]
            - float(f("ld_b")[0]) * emb_W.sum(1).astype(np.float32)[None, :])

    g = {"memb": np.ascontiguousarray(memb.reshape(KD, 128, DM).transpose(1, 0, 2)),
         "wpos": np.ascontiguousarray(wpos.astype(np.float32)),
         "ident": np.eye(128, dtype=np.float32)}

    s1 = np.float32(1.0 / np.sqrt(1.0 + EPS))
    def stack(fn):
        return np.ascontiguousarray(np.stack([fn(l) for l in range(L)]).astype(np.float32))

    def shuf(a):
        # (k*128, n) -> (128, k, n): SBUF layout with contiguous per-partition rows
        kn, n = a.shape
        return a.reshape(kn // 128, 128, n).transpose(1, 0, 2)

    g["vw1t"] = stack(lambda l: shuf(f("vc_W1")[l].T))
    g["vb1"] = stack(lambda l: f("vc_b1")[l].reshape(KH, 128).T)
    g["vw2t"] = stack(lambda l: shuf(f("vc_W2")[l].T))
    g["vb2"] = stack(lambda l: f("vc_b2")[l])
    g["aw1t"] = stack(lambda l: shuf(f("aa_W1")[l].T))
    g["ab1"] = stack(lambda l: f("aa_b1")[l].reshape(KH, 128).T)
    g["aw2t"] = stack(lambda l: shuf(f("aa_W2")[l].T))
    g["ab2"] = stack(lambda l: f("aa_b2")[l])
    g["m1"] = stack(lambda l: shuf(f("aa_Wq")[l].astype(np.float64).T @ f("aa_Wk")[l].astype(np.float64)))
    g["c1"] = stack(lambda l: f("aa_bq")[l].astype(np.float64) @ f("aa_Wk")[l].astype(np.float64))
    g["m2"] = stack(lambda l: shuf((f("aa_Wo")[l].astype(np.float64) @ f("aa_Wv")[l].astype(np.float64)).T))
    g["c2"] = stack(lambda l: f("aa_bv")[l].astype(np.float64) @ f("aa_Wo")[l].astype(np.float64).T
                    + f("aa_bo")[l].astype(np.float64))
    g["vsb"] = stack(lambda l: f("vc_ln_b")[l].sum(keepdims=True))
    g["asb"] = stack(lambda l: f("aa_ln_b")[l].sum(keepdims=True))
    g["vgc"] = stack(lambda l: f("vc_bn_g")[l] * s1)
    g["vbc"] = stack(lambda l: f("vc_bn_b")[l])
    g["vgl"] = stack(lambda l: f("vc_ln_g")[l])
    g["vbl"] = stack(lambda l: f("vc_ln_b")[l])
    g["agc"] = stack(lambda l: f("aa_bn_g")[l] * s1)
    g["abc"] = stack(lambda l: f("aa_bn_b")[l])
    g["agl"] = stack(lambda l: f("aa_ln_g")[l])
    g["abl"] = stack(lambda l: f("aa_ln_b")[l])
    return g


def kernel(**inputs):
    from concourse.bass_utils import run_bass_kernel_spmd
    nc = _compile()
    g = _host_prep(inputs)
    inp = np.asarray(inputs["inp"], np.float32)
    in_maps = []
    for core in range(NC_):
        m = dict(g)
        sl = inp[core * BPC:(core + 1) * BPC]          # (BPC, T, C)
        m["xin"] = np.ascontiguousarray(
            sl.reshape(BPC, KD, 128, C).transpose(2, 0, 1, 3))
        in_maps.append(m)
    res = run_bass_kernel_spmd(nc, in_maps, core_ids=list(range(NC_)))
    if res.exec_time_ns is not None:
        kernel.last_exec_time_ns = res.exec_time_ns
    out = np.concatenate([res.results[k]["out"] for k in range(NC_)], axis=0)
    return out


kernel.last_exec_time_ns = None


# revision 29
# speedup vs baseline: 1.3368x; 1.0060x over previous
"""CAWformer forward on 8 TRN2 NeuronCores — data parallel over batch.

Math notes (all exact algebraic rewrites of the reference):
  * irfft(xf_i * conj(xf_j)).mean(-1) == s_i * s_j / DM with s = x.sum(-1),
    so the FFT cross-correlation attention is softmax(outer(s, s)/c) @ x.
  * The 8-shift auto-attention: scores_i = <q@Wk, roll_i(x)> (+const that
    cancels in softmax); out = (sum_i p_i roll_i(x)) @ Wv.T @ Wo.T + const.
  * The depthwise smoothing conv is a (T,T) band matrix S; residual embed
    folds to inp[b].T @ (R.T @ emb_W.T) with R = I - S.
Per-core layout: rows = 2 batches x 128 channels, processed as two
128-partition chunks. Activations live rows-major (channels on partitions,
d_model on free axis); matmul operands are built feature-major via PE
transposes / algebraic tricks.
"""

import os
import numpy as np

B, T, C, DM, L, P, KS = 16, 512, 128, 512, 3, 64, 25
EPS = 1e-5
NS = DM // P           # 8 circular shifts
NC_ = 8                # cores
BPC = B // NC_         # batches per core = 2
H = 2 * DM             # FFN hidden = 1024
KD = DM // 128         # 4 k-tiles over d_model
KH = H // 128          # 8 k-tiles over hidden

F32 = None  # set lazily (mybir.dt.float32)


def _build(nc, tile, mybir, bass):
    F32 = mybir.dt.float32
    R32 = mybir.dt.float32r
    AT = mybir.ActivationFunctionType
    ALU = mybir.AluOpType
    AX = mybir.AxisListType

    def mmr(out, lhsT, rhs, start, stop):
        # float32r: single-pass PE matmul (4x the fp32 rate at N>=256),
        # fp32 memory format, ~tf32 multiply precision, fp32 accumulate.
        # Operand tiles are allocated float32r end-to-end because the BIR
        # verifier requires fp32r matmult inputs be *produced* as fp32r.
        nc.tensor.matmul(out, lhsT, rhs, start=start, stop=stop)

    # ---------------- DRAM I/O ----------------
    d = {}
    def din(name, shape, dt_=None):
        d[name] = nc.dram_tensor(name, list(shape), dt_ or F32, kind="ExternalInput")
        return d[name]

    # weight layouts are pre-shuffled on host to (128, k, n) so every DMA
    # is 128 partitions x contiguous-per-partition (full-bandwidth descriptors)
    din("xin", (128, BPC, KD, C), R32)
    din("memb", (128, KD, DM), R32)
    din("wpos", (C, DM))
    din("ident", (128, 128), R32)
    din("vw1t", (L, 128, KD, H), R32); din("vb1", (L, 128, KH)); din("vw2t", (L, 128, KH, DM), R32); din("vb2", (L, DM))
    din("aw1t", (L, 128, KD, H), R32); din("ab1", (L, 128, KH)); din("aw2t", (L, 128, KH, DM), R32); din("ab2", (L, DM))
    din("m1", (L, 128, KD, DM), R32); din("c1", (L, DM))
    din("m2", (L, 128, KD, DM), R32); din("c2", (L, DM))
    din("vsb", (L, 1)); din("asb", (L, 1))
    din("vgc", (L, C)); din("vbc", (L, C)); din("vgl", (L, DM)); din("vbl", (L, DM))
    din("agc", (L, C)); din("abc", (L, C)); din("agl", (L, DM)); din("abl", (L, DM))
    out_d = nc.dram_tensor("out", [BPC, C, DM], F32, kind="ExternalOutput")

    def bc_ap(src, parts=128):
        # broadcast a DRAM vector AP across partitions
        return bass.AP(tensor=src.tensor, offset=src.offset,
                       ap=[[0, parts]] + [list(x) for x in src.ap])

    def col_ap(src):
        # DRAM vector (n,) -> (n,1) partition-major AP
        return bass.AP(tensor=src.tensor, offset=src.offset,
                       ap=[list(src.ap[0]), [0, 1]])

    with tile.TileContext(nc) as tc:
        import contextlib
        ctx = contextlib.ExitStack()
        with ctx:
            wp = ctx.enter_context(tc.tile_pool(name="wp", bufs=1))
            ap_ = ctx.enter_context(tc.tile_pool(name="ap", bufs=1))
            bcp = ctx.enter_context(tc.tile_pool(name="bcp", bufs=8))
            sp = ctx.enter_context(tc.tile_pool(name="sp", bufs=8))
            cp = ctx.enter_context(tc.tile_pool(name="cp", bufs=1))
            pbig = ctx.enter_context(tc.tile_pool(name="pbig", bufs=3, space="PSUM"))
            ph = ctx.enter_context(tc.tile_pool(name="ph", bufs=2, space="PSUM"))
            pt = ctx.enter_context(tc.tile_pool(name="pt", bufs=2, space="PSUM"))

            # ---------------- constants ----------------
            ident = cp.tile([128, 128], R32)
            nc.sync.dma_start(out=ident, in_=d["ident"].ap())
            epsc = cp.tile([128, 1], F32)
            nc.vector.memset(epsc, EPS)
            memb_sb = cp.tile([128, KD, DM], R32)
            nc.sync.dma_start(out=memb_sb, in_=d["memb"].ap())
            wpos_sb = cp.tile([128, DM], F32)
            nc.sync.dma_start(out=wpos_sb, in_=d["wpos"].ap())
            xin_sb = cp.tile([128, BPC, KD, C], R32)
            nc.sync.dma_start(out=xin_sb, in_=d["xin"].ap())

            # ---------------- embed:  x[c] = xin[c].T @ memb + wpos ----------------
            x_t = ap_.tile([128, BPC, DM], R32, tag="x", bufs=3)
            for c in range(BPC):
                x_ps = pbig.tile([128, DM], F32, tag="big")
                for k in range(KD):
                    mmr(x_ps, xin_sb[:, c, k, :], memb_sb[:, k, :],
                        start=(k == 0), stop=(k == KD - 1))
                nc.vector.tensor_add(x_t[:, c, :], x_ps, wpos_sb)

            inv_sqc = float(1.0 / np.sqrt(DM * np.sqrt(DM)))
            phase = os.environ.get("KPHASE", "full")
            srow_of = {}

            # ---------------- layers ----------------
            for l in range(L if phase == "full" else 1):
                if phase == "emb":
                    break
                # ---- layer weight loads ----
                vw1t = wp.tile([128, KD, H], R32, tag="vw1t")
                nc.sync.dma_start(out=vw1t, in_=d["vw1t"][l])
                vb1 = sp.tile([128, KH], F32, tag="vb1")
                nc.sync.dma_start(out=vb1, in_=d["vb1"][l])
                vw2t = wp.tile([128, KH, DM], R32, tag="vw2t")
                nc.sync.dma_start(out=vw2t, in_=d["vw2t"][l])
                aw1t = wp.tile([128, KD, H], R32, tag="aw1t")
                nc.sync.dma_start(out=aw1t, in_=d["aw1t"][l])
                ab1 = sp.tile([128, KH], F32, tag="ab1")
                nc.sync.dma_start(out=ab1, in_=d["ab1"][l])
                aw2t = wp.tile([128, KH, DM], R32, tag="aw2t")
                nc.sync.dma_start(out=aw2t, in_=d["aw2t"][l])
                m1 = wp.tile([128, KD, DM], R32, tag="m1")
                nc.sync.dma_start(out=m1, in_=d["m1"][l])
                m2 = wp.tile([128, KD, DM], R32, tag="m2")
                nc.sync.dma_start(out=m2, in_=d["m2"][l])

                vgc = sp.tile([128, 1], F32, tag="vgc")
                nc.gpsimd.dma_start(out=vgc, in_=col_ap(d["vgc"][l]))
                agc = sp.tile([128, 1], F32, tag="agc")
                nc.gpsimd.dma_start(out=agc, in_=col_ap(d["agc"][l]))
                vbc = sp.tile([128, 1], F32, tag="vbc")
                nc.gpsimd.dma_start(out=vbc, in_=col_ap(d["vbc"][l]))
                abc = sp.tile([128, 1], F32, tag="abc")
                nc.gpsimd.dma_start(out=abc, in_=col_ap(d["abc"][l]))
                vbcf = bcp.tile([128, 128], F32, tag="bc2", bufs=4, name=f"vbcf{l}")
                nc.gpsimd.dma_start(out=vbcf, in_=bc_ap(d["vbc"][l]))

                def bcast(name):
                    t = bcp.tile([128, DM], F32, tag="bc", name=f"{name}_bc{l}")
                    nc.gpsimd.dma_start(out=t, in_=bc_ap(d[name][l]))
                    return t
                c1b = bcast("c1"); c2b = bcast("c2")
                vb2b = bcast("vb2"); ab2b = bcast("ab2")
                vglb = bcast("vgl"); vblb = bcast("vbl")
                aglb = bcast("agl"); ablb = bcast("abl")

                # gcI = diag(gc_vc) as dense tile for the "+I" residual fold
                gcI = sp.tile([128, 128], F32, tag="gcI", bufs=2)
                nc.vector.tensor_scalar_mul(gcI, ident, vgc)
                vsb = sp.tile([128, 1], F32, tag="vsb")
                nc.gpsimd.dma_start(out=vsb, in_=bc_ap(d["vsb"][l]))
                asb = sp.tile([128, 1], F32, tag="asb")
                nc.gpsimd.dma_start(out=asb, in_=bc_ap(d["asb"][l]))

                # ============ VarCor block ============
                # s = rowsum(x) * 1/sqrt(DM*sqrt(DM)) (split sqrt per side)
                cT = ap_.tile([128, BPC, 128], R32, tag="cT")
                for c in range(BPC):
                    if c in srow_of:
                        srow = srow_of[c]
                    else:
                        srow = sp.tile([128, 1], F32, tag="srow", bufs=4)
                        nc.vector.tensor_reduce(srow, x_t[:, c, :], AX.X, ALU.add)
                    s2 = sp.tile([128, 1], R32, tag="s2", bufs=4)
                    nc.scalar.mul(s2, srow, inv_sqc)
                    sT_ps = pbig.tile([1, 128], R32, tag="big", name=f"sTps{l}_{c}")
                    nc.tensor.transpose(sT_ps, s2, ident)
                    sT = sp.tile([1, 128], R32, tag="sT", bufs=4)
                    nc.scalar.activation(sT, sT_ps, AT.Identity)
                    corr_ps = pbig.tile([128, 128], F32, tag="big", name=f"corrps{l}_{c}")
                    mmr(corr_ps, sT, sT, start=True, stop=True)
                    # softmax over free axis (values are O(1): skip max-sub)
                    # + BN row-scale + +I fold
                    corrE = ap_.tile([128, 128], F32, tag="corrE", bufs=2)
                    rsum = sp.tile([128, 1], F32, tag="rsum", bufs=4)
                    nc.scalar.activation(corrE, corr_ps, AT.Exp, accum_out=rsum)
                    rinv = sp.tile([128, 1], F32, tag="rinv", bufs=4)
                    nc.vector.reciprocal(rinv, rsum)
                    corrBN = ap_.tile([128, 128], R32, tag="corrBN", bufs=2)
                    nc.vector.tensor_scalar(corrBN, corrE, rinv, vgc, ALU.mult, ALU.mult)
                    nc.vector.tensor_add(corrBN, corrBN, gcI)
                    cT_ps = pt.tile([128, 128], R32, tag="t", name=f"cTps{l}_{c}")
                    nc.tensor.transpose(cT_ps, corrBN, ident)
                    nc.vector.tensor_copy(cT[:, c, :], cT_ps)

                # r2 rows-major and feature-major via two matmul sets
                r2r = ap_.tile([128, BPC, DM], R32, tag="r2r")
                r2T = ap_.tile([128, KD, 2 * 128], R32, tag="r2T")
                for c in range(BPC):
                    rr_ps = pbig.tile([128, DM], F32, tag="big", name=f"rrps{l}_{c}")
                    mmr(rr_ps, cT[:, c, :], x_t[:, c, :], start=True, stop=True)
                    nc.scalar.activation(r2r[:, c, :], rr_ps, AT.Identity, bias=vbc)
                    for m in range(KD):
                        rt_ps = pt.tile([128, 128], F32, tag="t", name=f"rtps{l}_{c}_{m}")
                        mmr(rt_ps, x_t[:, c, m * 128:(m + 1) * 128],
                            cT[:, c, :], start=True, stop=True)
                        # feature-major r2T: BN beta is along the free (channel)
                        # axis here, so add it via a partition-broadcast tile
                        nc.vector.tensor_add(r2T[:, m, c * 128:(c + 1) * 128],
                                             rt_ps, vbcf)

                if phase == "corr":
                    x_t = r2r
                    break
                x_t = _ffn_ln(nc, tile, mybir, bass, tc, ap_, sp, bcp, ph, pbig,
                              r2T, r2r, vw1t, vb1, vw2t, vb2b, vglb, vblb, l, "v", epsc,
                              vsb, inv_sqc, srow_of)
                if phase == "vc0":
                    break

                # ============ Auto-attention block ============
                # xT feature-major
                xT = ap_.tile([128, KD, 2 * 128], R32, tag="xT")
                for c in range(BPC):
                    for m in range(KD):
                        tp = pt.tile([128, 128], R32, tag="t", name=f"xTps{l}_{c}_{m}")
                        nc.tensor.transpose(tp, x_t[:, c, m * 128:(m + 1) * 128], ident)
                        nc.vector.tensor_copy(xT[:, m, c * 128:(c + 1) * 128], tp)

                # u = x @ M1 + c1   (rows-major out)
                u_t = ap_.tile([128, BPC, DM], F32, tag="u")
                for c in range(BPC):
                    u_ps = pbig.tile([128, DM], F32, tag="big", name=f"ups{l}_{c}")
                    for k in range(KD):
                        mmr(u_ps, xT[:, k, c * 128:(c + 1) * 128],
                            m1[:, k, :], start=(k == 0), stop=(k == KD - 1))
                    nc.vector.tensor_add(u_t[:, c, :], u_ps, c1b)

                if phase == "u":
                    x_t = u_t
                    break

                # scores S[r,i] = <u, roll_i(x)> * DM^-0.5 ; softmax over i
                scl = float(DM ** -0.5)
                Sp_t = ap_.tile([128, BPC, NS], F32, tag="Sp")
                trash = ap_.tile([128, DM], F32, tag="trash", bufs=1)
                for c in range(BPC):
                    # NOTE: tensor_tensor_reduce wedges the device on this
                    # walrus/NRT build (NRT_EXEC_UNIT_UNRECOVERABLE); use
                    # scalar_tensor_tensor's accum_out instead.
                    Sa = sp.tile([128, NS], F32, tag="Sa", bufs=2)
                    Sb = sp.tile([128, NS], F32, tag="Sb", bufs=2)
                    nc.vector.memset(Sb[:, 0:1], 0.0)
                    for i in range(NS):
                        sh = P * i
                        if sh == 0:
                            nc.vector.scalar_tensor_tensor(
                                out=trash, in0=u_t[:, c, :], scalar=scl,
                                in1=x_t[:, c, :], op0=ALU.mult, op1=ALU.mult,
                                accum_out=Sa[:, i:i + 1])
                        else:
                            nc.vector.scalar_tensor_tensor(
                                out=trash[:, :DM - sh], in0=u_t[:, c, :DM - sh],
                                scalar=scl, in1=x_t[:, c, sh:],
                                op0=ALU.mult, op1=ALU.mult, accum_out=Sa[:, i:i + 1])
                            nc.vector.scalar_tensor_tensor(
                                out=trash[:, DM - sh:], in0=u_t[:, c, DM - sh:],
                                scalar=scl, in1=x_t[:, c, :sh],
                                op0=ALU.mult, op1=ALU.mult, accum_out=Sb[:, i:i + 1])
                    S = sp.tile([128, NS], F32, tag="S", bufs=2)
                    nc.vector.tensor_add(S, Sa, Sb)
                    Se = sp.tile([128, NS], F32, tag="Se", bufs=2)
                    ssum = sp.tile([128, 1], F32, tag="ssum", bufs=4)
                    nc.scalar.activation(Se, S, AT.Exp, accum_out=ssum)
                    sinv = sp.tile([128, 1], F32, tag="sinv", bufs=4)
                    nc.vector.reciprocal(sinv, ssum)
                    nc.vector.tensor_scalar(Sp_t[:, c, :], Se, sinv, None, ALU.mult)

                if phase == "sc":
                    x_t = ap_.tile([128, BPC, DM], F32, tag="scdump")
                    nc.vector.memset(x_t, 0.0)
                    for c in range(BPC):
                        nc.vector.tensor_copy(x_t[:, c, 0:NS], Sp_t[:, c, :])
                    break

                # vm = sum_i p_i roll_i(x) via diag matmuls accumulating in PSUM
                vm_t = ap_.tile([128, BPC, DM], R32, tag="vm")
                for c in range(BPC):
                    vm_ps = pbig.tile([128, DM], F32, tag="big", name=f"vmps{l}_{c}")
                    for i in range(NS):
                        dg = ap_.tile([128, 128], R32, tag="dg", bufs=3)
                        nc.vector.tensor_scalar_mul(dg, ident, Sp_t[:, c, i:i + 1])
                        sh = P * i
                        last = (i == NS - 1)
                        if sh == 0:
                            mmr(vm_ps, dg, x_t[:, c, :], start=True, stop=False)
                        else:
                            mmr(vm_ps[:, :DM - sh], dg, x_t[:, c, sh:],
                                start=False, stop=False)
                            mmr(vm_ps[:, DM - sh:], dg, x_t[:, c, :sh],
                                start=False, stop=last)
                    nc.vector.tensor_copy(vm_t[:, c, :], vm_ps)

                if phase == "vm":
                    x_t = vm_t
                    break

                # vmT feature-major
                vmT = ap_.tile([128, KD, 2 * 128], R32, tag="vmT")
                for c in range(BPC):
                    for m in range(KD):
                        tp2 = pt.tile([128, 128], R32, tag="t", name=f"vmTps{l}_{c}_{m}")
                        nc.tensor.transpose(tp2, vm_t[:, c, m * 128:(m + 1) * 128], ident)
                        nc.vector.tensor_copy(vmT[:, m, c * 128:(c + 1) * 128], tp2)

                # attn out rows-major: o = vm @ M2 + c2 ; r1 = BN(o + x)
                r1r = ap_.tile([128, BPC, DM], R32, tag="r1r")
                for c in range(BPC):
                    o_ps = pbig.tile([128, DM], F32, tag="big", name=f"ops{l}_{c}")
                    for k in range(KD):
                        mmr(o_ps, vmT[:, k, c * 128:(c + 1) * 128],
                            m2[:, k, :], start=(k == 0), stop=(k == KD - 1))
                    t1 = ap_.tile([128, DM], F32, tag="t1", bufs=2)
                    nc.vector.tensor_add(t1, o_ps, x_t[:, c, :])
                    nc.vector.tensor_add(t1, t1, c2b)
                    nc.scalar.activation(r1r[:, c, :], t1, AT.Identity, bias=abc, scale=agc)

                if phase == "attn":
                    x_t = r1r
                    break

                # r1T feature-major
                r1T = ap_.tile([128, KD, 2 * 128], R32, tag="r1T")
                for c in range(BPC):
                    for m in range(KD):
                        tp3 = pt.tile([128, 128], R32, tag="t", name=f"r1Tps{l}_{c}_{m}")
                        nc.tensor.transpose(tp3, r1r[:, c, m * 128:(m + 1) * 128], ident)
                        nc.vector.tensor_copy(r1T[:, m, c * 128:(c + 1) * 128], tp3)

                x_t = _ffn_ln(nc, tile, mybir, bass, tc, ap_, sp, bcp, ph, pbig,
                              r1T, r1r, aw1t, ab1, aw2t, ab2b, aglb, ablb, l, "a", epsc,
                              asb, inv_sqc, srow_of)

            # ---------------- store ----------------
            for c in range(BPC):
                nc.sync.dma_start(out=out_d.ap()[c], in_=x_t[:, c, :].bitcast(F32))


def _ffn_ln(nc, tile, mybir, bass, tc, ap_, sp, bcp, ph, pbig,
            rT, rrows, w1t, b1, w2t, b2b, glb, blb, l, pfx, epsc,
            sumb, inv_sqc, srow_of):
    """h = gelu(r @ W1.T + b1); y = h @ W2.T + b2; x = LN(y + r) * g + b.

    Also emits (for the "a" blocks feeding the next varcor) the row-sum of
    the next x via <xn, g> + sum(b) so the correlation chain never waits on
    the gamma/beta affine (which runs on GpSimd off the critical path)."""
    F32 = mybir.dt.float32
    R32 = mybir.dt.float32r
    AT = mybir.ActivationFunctionType
    ALU = mybir.AluOpType
    AX = mybir.AxisListType

    hT = ap_.tile([128, KH, 2 * 128], R32, tag="hT", bufs=2, name=f"hT{pfx}{l}")
    for mh2 in range(KH // 2):
        h_ps = ph.tile([128, 2, 128 * 2], F32, tag="h", name=f"hps{pfx}{l}_{mh2}")
        for half in range(2):
            mh = mh2 * 2 + half
            for k in range(KD):
                nc.tensor.matmul(h_ps[:, half, :], w1t[:, k, mh * 128:(mh + 1) * 128],
                                 rT[:, k, :], start=(k == 0), stop=(k == KD - 1))
            nc.scalar.activation(hT[:, mh, :], h_ps[:, half, :], AT.Gelu,
                                 bias=b1[:, mh:mh + 1])
    # prewarm the Sqrt activation table while FFN2 runs so the LN-critical
    # Sqrt below hits a warm table (ACT table switches cost ~1.3us)
    stdw = sp.tile([128, 1], F32, tag="std", bufs=4, name=f"stdw{pfx}{l}")
    nc.scalar.activation(stdw, epsc, AT.Sqrt, bias=epsc)

    x_new = ap_.tile([128, BPC, DM], R32, tag="x", bufs=3, name=f"x{pfx}{l}")
    for c in range(BPC):
        y_ps = pbig.tile([128, DM], F32, tag="big", name=f"yps{pfx}{l}_{c}")
        for k in range(KH):
            nc.tensor.matmul(y_ps, hT[:, k, c * 128:(c + 1) * 128],
                             w2t[:, k, :], start=(k == 0), stop=(k == KH - 1))
        rb = ap_.tile([128, DM], F32, tag="rb", bufs=2, name=f"rb{pfx}{l}_{c}")
        nc.vector.tensor_add(rb, rrows[:, c, :], b2b)
        z = ap_.tile([128, DM], F32, tag="z", bufs=2, name=f"z{pfx}{l}_{c}")
        nc.vector.tensor_add(z, y_ps, rb)
        st6 = sp.tile([128, 6], F32, tag="st6", bufs=4)
        nc.vector.bn_stats(out=st6, in_=z)
        mv = sp.tile([128, 2], F32, tag="mv", bufs=4)
        nc.vector.bn_aggr(out=mv, in_=st6)
        std = sp.tile([128, 1], F32, tag="std", bufs=4)
        nc.scalar.activation(std, mv[:, 1:2], AT.Sqrt, bias=epsc)
        rstd = sp.tile([128, 1], F32, tag="rstd", bufs=4)
        nc.vector.reciprocal(rstd, std)
        nb = sp.tile([128, 1], F32, tag="nb", bufs=4)
        nc.vector.tensor_scalar_mul(nb, mv[:, 0:1], -1.0)
        xn = ap_.tile([128, DM], F32, tag="xn", bufs=2, name=f"xn{pfx}{l}_{c}")
        nc.vector.tensor_scalar(xn, z, nb, rstd, ALU.add, ALU.mult)
        if pfx == "a" and l < L - 1:
            # next-layer corr row-sum: <xn*rstd-normalized x, g> + sum(b)
            trash2 = ap_.tile([128, DM], F32, tag="trash", bufs=1,
                              name=f"tr2{pfx}{l}_{c}")
            sraw = sp.tile([128, 1], F32, tag="sraw", bufs=4)
            nc.vector.scalar_tensor_tensor(
                out=trash2, in0=xn, scalar=1.0, in1=glb,
                op0=ALU.mult, op1=ALU.mult, accum_out=sraw)
            srow = sp.tile([128, 1], F32, tag="srow", bufs=4, name=f"srow{pfx}{l}_{c}")
            nc.scalar.activation(srow, sraw, AT.Identity, bias=sumb)
            srow_of[c] = srow
        # affine (the next-layer corr chain does not wait on it: srow above)
        nc.vector.tensor_mul(x_new[:, c, :], xn, glb)
        nc.vector.tensor_add(x_new[:, c, :], x_new[:, c, :], blb)
    return x_new


# ======================================================================
# host side
# ======================================================================

_COMPILED = {}


def _compile():
    if "nc" in _COMPILED:
        return _COMPILED["nc"]
    import concourse.bass as bass
    import concourse.bacc as bacc
    import concourse.tile as tile
    from concourse import mybir
    nc = bacc.Bacc("TRN2", target_bir_lowering=False, debug=False, num_devices=NC_)
    _build(nc, tile, mybir, bass)
    nc.compile()
    _COMPILED["nc"] = nc
    return nc


def _host_prep(inputs):
    f = lambda k: np.asarray(inputs[k], np.float32)
    ld_w = f("ld_w").reshape(KS).astype(np.float64)
    # conv matrix with replicate padding, R = I - S
    S = np.zeros((T, T), np.float64)
    idx = np.clip(np.arange(T)[:, None] + np.arange(KS)[None, :] - KS // 2, 0, T - 1)
    for k in range(KS):
        np.add.at(S, (np.arange(T), idx[:, k]), ld_w[k])
    Rm = np.eye(T) - S
    emb_W = f("emb_W").astype(np.float64)
    memb = (Rm.T @ emb_W.T).astype(np.float32)              # (T, DM)
    wpos = (f("W_pos") + f("emb_b")[None, :]
            - float(f("ld_b")[0]) * emb_W.sum(1).astype(np.float32)[None, :])

    g = {"memb": np.ascontiguousarray(memb.reshape(KD, 128, DM).transpose(1, 0, 2)),
         "wpos": np.ascontiguousarray(wpos.astype(np.float32)),
         "ident": np.eye(128, dtype=np.float32)}

    s1 = np.float32(1.0 / np.sqrt(1.0 + EPS))
    def stack(fn):
        return np.ascontiguousarray(np.stack([fn(l) for l in range(L)]).astype(np.float32))

    def shuf(a):
        # (k*128, n) -> (128, k, n): SBUF layout with contiguous per-partition rows
        kn, n = a.shape
        return a.reshape(kn // 128, 128, n).transpose(1, 0, 2)

    g["vw1t"] = stack(lambda l: shuf(f("vc_W1")[l].T))
    g["vb1"] = stack(lambda l: f("vc_b1")[l].reshape(KH, 128).T)
    g["vw2t"] = stack(lambda l: shuf(f("vc_W2")[l].T))
    g["vb2"] = stack(lambda l: f("vc_b2")[l])
    g["aw1t"] = stack(lambda l: shuf(f("aa_W1")[l].T))
    g["ab1"] = stack(lambda l: f("aa_b1")[l].reshape(KH, 128).T)
    g["aw2t"] = stack(lambda l: shuf(f("aa_W2")[l].T))
    g["ab2"] = stack(lambda l: f("aa_b2")[l])
    g["m1"] = stack(lambda l: shuf(f("aa_Wq")[l].astype(np.float64).T @ f("aa_Wk")[l].astype(np.float64)))
    g["c1"] = stack(lambda l: f("aa_bq")[l].astype(np.float64) @ f("aa_Wk")[l].astype(np.float64))
    g["m2"] = stack(lambda l: shuf((f("aa_Wo")[l].astype(np.float64) @ f("aa_Wv")[l].astype(np.float64)).T))
    g["c2"] = stack(lambda l: f("aa_bv")[l].astype(np.float64) @ f("aa_Wo")[l].astype(np.float64).T
                    + f("aa_bo")[l].astype(np.float64))
    g["vsb"] = stack(lambda l: f("vc_ln_b")[l].sum(keepdims=True))
    g["asb"] = stack(lambda l: f("aa_ln_b")[l].sum(keepdims=True))
    g["vgc"] = stack(lambda l: f("vc_bn_g")[l] * s1)
    g["vbc"] = stack(lambda l: f("vc_bn_b")[l])
    g["vgl"] = stack(lambda l: f("vc_ln_g")[l])
    g["vbl"] = stack(lambda l: f("vc_ln_b")[l])
    g["agc"] = stack(lambda l: f("aa_bn_g")[l] * s1)
    g["abc"] = stack(lambda l: f("aa_bn_b")[l])
    g["agl"] = stack(lambda l: f("aa_ln_g")[l])
    g["abl"] = stack(lambda l: f("aa_ln_b")[l])
    return g


def kernel(**inputs):
    from concourse.bass_utils import run_bass_kernel_spmd
    nc = _compile()
    g = _host_prep(inputs)
    inp = np.asarray(inputs["inp"], np.float32)
    in_maps = []
    for core in range(NC_):
        m = dict(g)
        sl = inp[core * BPC:(core + 1) * BPC]          # (BPC, T, C)
        m["xin"] = np.ascontiguousarray(
            sl.reshape(BPC, KD, 128, C).transpose(2, 0, 1, 3))
        in_maps.append(m)
    res = run_bass_kernel_spmd(nc, in_maps, core_ids=list(range(NC_)))
    if res.exec_time_ns is not None:
        kernel.last_exec_time_ns = res.exec_time_ns
    out = np.concatenate([res.results[k]["out"] for k in range(NC_)], axis=0)
    return out


kernel.last_exec_time_ns = None
